# revision 1
# baseline (speedup 1.0000x reference)
"""Trainium2 Bass kernel for the truncated-spectrum 2D conv (CF2DConv).

Math: out = iDCT_y( irfft_x( mix_per_mode( rfft_x( DCT_y(x) )[:64,:64] ) ) )
All transforms are dense truncated matrices; the whole op is a chain of
matmuls plus a per-mode complex channel mix.

Execution: 3 SPMD launches on 8 NeuronCores.
  phase 1  (shard (b, nx-half)): partial forward spectrum per core
  phase 2a (shard a-modes):      per-mode complex mix, R read exactly once
  phase 2b (shard (b, nx-half)): inverse transforms, contiguous output writes
Host does the (cheap, few-MB) re-shards between launches.
"""
import numpy as np
from contextlib import ExitStack

import concourse.bass as bass
import concourse.mybir as mybir
import concourse.tile as tile
from concourse.bass_utils import run_bass_kernel_spmd

B, NX, NY, DV = 4, 512, 512, 32
KX, KY = 64, 64
NCORES = 8
NXH = NX // 2          # 256 rows per (b, h) core
F32 = mybir.dt.float32
F32R = mybir.dt.float32r

def _split_multiwait(nc):
    """Each 64B engine instruction has ONE sync-wait slot; Tile can attach
    several (e.g. two operands arriving on different DMAHW sem lanes), which
    walrus codegen rejects ("Too many sync wait commands"). Spill excess
    waits (and updates) onto chains of single-wait no-ops on the same
    engine queue."""
    cnt = 0
    for fn in nc.m.functions:
        for blk in fn.blocks:
            insts = list(blk.instructions)
            out = []
            changed = False
            for inst in insts:
                si = inst.sync_info
                if si is not None:
                    waits = list(si.on_wait or [])
                    ups = list(si.on_update or [])
                    if len(waits) > 1:
                        for w in waits[:-1]:
                            cnt += 1
                            out.append(mybir.InstNoOp(
                                name=f"premw{cnt}_{inst.name}",
                                sync_info=mybir.SyncInfo(on_wait=[w],
                                                         on_update=[]),
                                bass_nofuse=True, engine=inst.engine))
                        inst.sync_info = mybir.SyncInfo(
                            on_wait=waits[-1:], on_update=ups)
                        changed = True
                    if len(ups) > 1:
                        inst.sync_info = mybir.SyncInfo(
                            on_wait=list(inst.sync_info.on_wait or []),
                            on_update=ups[:1])
                        out.append(inst)
                        for u in ups[1:]:
                            cnt += 1
                            out.append(mybir.InstNoOp(
                                name=f"postmw{cnt}_{inst.name}",
                                sync_info=mybir.SyncInfo(on_wait=[],
                                                         on_update=[u]),
                                bass_nofuse=True, engine=inst.engine))
                        changed = True
                        continue
                out.append(inst)
            if changed:
                blk.instructions = out
    return nc


# ----------------------------------------------------------------------------
# Host-side constant transform matrices (float64 -> float32)
# ----------------------------------------------------------------------------


def _copy(nc, idx, out, in_):
    if idx % 2 == 0:
        nc.scalar.copy(out, in_)
    else:
        nc.vector.tensor_copy(out, in_)


def _build_consts():
    ny = np.arange(NY)
    m = np.arange(KY)
    Cy = np.cos(np.pi * (2 * ny[None, :] + 1) * m[:, None] / (2 * NY))
    s = np.full((KY, 1), np.sqrt(2.0 / NY)); s[0, 0] = np.sqrt(1.0 / NY)
    Cy = Cy * s                                     # [KY, NY]

    nx = np.arange(NX)
    a = np.arange(KX)
    ang = 2 * np.pi * a[:, None] * nx[None, :] / NX
    Fre = np.cos(ang) / np.sqrt(NX)                 # [KX, NX]
    Fim = -np.sin(ang) / np.sqrt(NX)

    w = np.full(KX, 2.0); w[0] = 1.0
    Gr = w[None, :] * np.cos(ang.T) / np.sqrt(NX)   # [NX, KX]
    Gi = -w[None, :] * np.sin(ang.T) / np.sqrt(NX)

    # FxT_all [NX, 128]: cols 0:64 = Fre^T, 64:128 = Fim^T
    FxT = np.concatenate([Fre.T, Fim.T], axis=1)    # [512, 128]
    CyT = np.concatenate([Cy.T, np.zeros((NY, 64))], axis=1)  # [512, 128] zero-padded
    # G_all [128(alpha), NX]: rows 0:64 = Gr^T, 64:128 = Gi^T
    G = np.concatenate([Gr.T, Gi.T], axis=0)        # [128, 512]
    return (FxT.astype(np.float32), CyT.astype(np.float32),
            G.astype(np.float32), Cy.astype(np.float32))


_FXT, _CYT, _G, _CY = _build_consts()
_EYE = np.eye(128, dtype=np.float32)


# ----------------------------------------------------------------------------
# Phase 1: x[b, h*256:(h+1)*256] -> partial truncated spectrum
#   in : xh   [256, NY*DV]  (fp32, viewed fp32r)
#        fxt  [256, 128]    FxT rows for this nx-half
#        cyt  [512, 64]     CyT
#   out: xtr  [128, 2048]   layout [alpha, (j, m)]   (partial: sum over h needed)
# ----------------------------------------------------------------------------
def build_phase1():
    nc = bass.Bass()
    xh = nc.declare_dram_parameter("xh", [NXH, NY * DV], F32R, isOutput=False)
    fxt = nc.declare_dram_parameter("fxt", [NXH, 128], F32R, isOutput=False)
    cyt = nc.declare_dram_parameter("cyt", [NY, 128], F32R, isOutput=False)
    xtr = nc.declare_dram_parameter("xtr", [64, DV * 128], F32, isOutput=True)
    id_ext = nc.declare_dram_parameter("ident", [128, 128], F32R, isOutput=False)

    with ExitStack() as ctx:
        tc = ctx.enter_context(tile.TileContext(nc))
        consts = ctx.enter_context(tc.tile_pool(name="consts", bufs=1))
        xpool = ctx.enter_context(tc.tile_pool(name="xpool", bufs=8))
        t1pool = ctx.enter_context(tc.tile_pool(name="t1pool", bufs=1))
        t1tpool = ctx.enter_context(tc.tile_pool(name="t1tpool", bufs=2))
        outpool = ctx.enter_context(tc.tile_pool(name="outpool", bufs=1))
        psB = ctx.enter_context(tc.tile_pool(name="psB", bufs=4, space="PSUM"))
        psT = ctx.enter_context(tc.tile_pool(name="psT", bufs=2, space="PSUM"))
        psA = ctx.enter_context(tc.tile_pool(name="psA", bufs=2, space="PSUM"))

        fxt_t = consts.tile([128, 256], F32R)       # [p, (k,alpha)]
        for k in range(2):
            nc.sync.dma_start(out=fxt_t[:, k * 128:(k + 1) * 128],
                              in_=fxt[k * 128:(k + 1) * 128, :])
        cyt_t = consts.tile([128, 512], F32R)       # [p, (q, m-pad128)]
        for q in range(4):
            nc.sync.dma_start(out=cyt_t[:, q * 128:(q + 1) * 128],
                              in_=cyt[q * 128:(q + 1) * 128, :])
        ident = consts.tile([128, 128], F32R)
        nc.sync.dma_start(out=ident, in_=id_ext[:, :])

        T1qs = [t1pool.tile([128, NY * DV // 4], F32R, tag=f"T1q{q}", bufs=1,
                            name=f"T1q{q}") for q in range(4)]  # [alpha, (ny-q, j)]

        # ---- stage B: FFT-X (contract nx); t-tiles grouped per weight
        # switch (first groups small so the first matmul starts early) ----
        GROUPS = [[0], [1, 2], [3, 4, 5]] + [
            list(range(6 + 4 * g, 6 + 4 * (g + 1))) for g in range(6)] + [[30, 31]]
        for grp in GROUPS:
            xts = []
            for t in grp:
                xt = xpool.tile([128, 1024], F32R, tag="xt", name=f"xt{t}")
                for k in range(2):
                    nc.sync.dma_start(
                        out=xt[:, k * 512:(k + 1) * 512],
                        in_=xh[k * 128:(k + 1) * 128, t * 512:(t + 1) * 512])
                xts.append(xt)
            pBs = [psB.tile([128, 512], F32, tag=f"pB{tt}", bufs=1,
                            name=f"pB{tt}_{grp[0]}") for tt in range(len(grp))]
            for k in range(2):
                for tt in range(len(grp)):
                    nc.tensor.matmul(pBs[tt], fxt_t[:, k * 128:(k + 1) * 128],
                                     xts[tt][:, k * 512:(k + 1) * 512],
                                     start=(k == 0), stop=(k == 1))
            for tt, t in enumerate(grp):
                _copy(nc, t, T1qs[t // 8][:, (t % 8) * 512:(t % 8 + 1) * 512],
                      pBs[tt].bitcast(F32R))

        # ---- transposes (per ny-128-block q); 4 j-planes per psum bank ----
        T1ts = []
        for q in range(4):
            T1v = T1qs[q].rearrange("p (ny j) -> p ny j", j=DV)
            T1t = t1tpool.tile([128, DV * 128], F32R, tag=f"T1t{q}", bufs=1,
                               name=f"T1t{q}")   # [ny128, (j, alpha)]
            T1ts.append(T1t)
            for jg in range(8):
                pT = psT.tile([128, 512], F32R, tag="pT", name=f"pT{q}_{jg}")
                for jj in range(4):
                    j = jg * 4 + jj
                    nc.tensor.transpose(pT[:, jj * 128:(jj + 1) * 128],
                                        T1v[:, :, j], ident)
                _copy(nc, jg, T1t[:, jg * 512:(jg + 1) * 512], pT)

        # ---- stage A (DCT-Y, contract ny): out rows 0:64 = m, 64:128 = pad ----
        acc = [psA.tile([128, 512], F32, tag=f"acc{i}", bufs=1, name=f"acc{i}")
               for i in range(2)]
        for quarter in range(4):
            xq = outpool.tile([128, 1024], F32R, tag=f"xq{quarter}", bufs=1,
                              name=f"xq{quarter}")   # rows 0:64 = [m, (j,alpha)-q]
            for q in range(4):
                for n in range(2):
                    nc.tensor.matmul(
                        acc[n],
                        cyt_t[:, q * 128:(q + 1) * 128],
                        T1ts[q][:, quarter * 1024 + n * 512:
                                quarter * 1024 + (n + 1) * 512],
                        start=(q == 0), stop=(q == 3))
            for n in range(2):
                _copy(nc, n, xq[0:64, n * 512:(n + 1) * 512], acc[n][0:64, :])
            nc.sync.dma_start(out=xtr[:, quarter * 1024:(quarter + 1) * 1024],
                              in_=xq[0:64, :].bitcast(F32))
    return _split_multiwait(nc)


# ----------------------------------------------------------------------------
# Phase 2a: per-mode complex channel mix, sharded over a (8 a-values per core)
#   in : w    [64, 32*8*64]  [(rr/ri, j), (i, a_l, m)]  R slice, fp32
#        xre  [64, 8*64*4]   rows (xr | -xi), cols (a_l, m, b)
#        xim  [64, 8*64*4]   rows (xi |  xr), cols (a_l, m, b)
#   out: y    [64, 8*64*4]   [(q, i), (a_l, m, b)]
# ----------------------------------------------------------------------------
def build_phase2a():
    NMODE = (KX // NCORES) * KY                      # 512 modes per core
    NG = NMODE // 2                                  # 256 mode-pair groups
    nc = bass.Bass()
    # w2: per group g a [128, 64] block-diag lhsT; rows (u2, rr/ri, j),
    #     cols (u2, i32); concatenated along free -> [128, 256*64]
    w2 = nc.declare_dram_parameter("w2", [128, NG * 64], F32R, isOutput=False)
    # x2: per group g a [128, 8] rhs; cols (q2, b4) where q=0 -> re-out
    #     (rows: xr | -xi per u-block), q=1 -> im-out (xi | xr)
    x2 = nc.declare_dram_parameter("x2", [128, NG * 8], F32R, isOutput=False)
    # y: [64 = (u2, i32), (g, q2, b4)]
    y = nc.declare_dram_parameter("y", [64, NMODE * B], F32, isOutput=True)

    with ExitStack() as ctx:
        tc = ctx.enter_context(tile.TileContext(nc))
        consts = ctx.enter_context(tc.tile_pool(name="consts", bufs=1))
        outpool = ctx.enter_context(tc.tile_pool(name="outpool", bufs=1))
        psY = ctx.enter_context(tc.tile_pool(name="psY", bufs=4, space="PSUM"))

        x_ts = []
        for c in range(2):
            x_c = consts.tile([128, NG * 4], F32R, tag=f"x{c}", name=f"x{c}")
            nc.sync.dma_start(out=x_c, in_=x2[:, c * NG * 4:(c + 1) * NG * 4])
            x_ts.append(x_c)
        w_ts = []
        for c in range(8):
            w_c = consts.tile([128, 2048], F32R, tag=f"w{c}", name=f"w{c}")
            nc.sync.dma_start(out=w_c, in_=w2[:, c * 2048:(c + 1) * 2048])
            w_ts.append(w_c)
        y_ts = [outpool.tile([64, 512], F32, tag=f"y{bk}", name=f"y{bk}")
                for bk in range(4)]

        for bk in range(4):                          # 64 groups per psum bank
            pY = psY.tile([64, 512], F32)
            for gg in range(64):
                g = bk * 64 + gg
                nc.tensor.matmul(pY[:, gg * 8:(gg + 1) * 8],
                                 w_ts[g // 32][:, (g % 32) * 64:
                                               (g % 32 + 1) * 64],
                                 x_ts[g // 128][:, (g % 128) * 8:
                                                (g % 128 + 1) * 8],
                                 start=True, stop=True)
            _copy(nc, bk, y_ts[bk], pY)
            nc.sync.dma_start(out=y[:, bk * 512:(bk + 1) * 512], in_=y_ts[bk])
    return _split_multiwait(nc)


# ----------------------------------------------------------------------------
# Phase 2b: inverse transforms per (b, nx-half)
#   in : yb  [128, 2048]  [(q, a), (i, m)]
#        gh  [128, 256]   G rows alpha, cols nx-local
#        cym [64, 512]    Cy [m, ny]
#   out: oh  [256, NY*DV] rows nx-local, cols (ny, i)
# ----------------------------------------------------------------------------
def build_phase2b():
    nc = bass.Bass()
    yb = nc.declare_dram_parameter("yb", [128, DV * KY], F32R, isOutput=False)
    gh = nc.declare_dram_parameter("gh", [128, NXH], F32R, isOutput=False)
    cym = nc.declare_dram_parameter("cym", [KY, NY], F32R, isOutput=False)
    oh = nc.declare_dram_parameter("oh", [NXH, NY * DV], F32, isOutput=True)

    with ExitStack() as ctx:
        tc = ctx.enter_context(tile.TileContext(nc))
        consts = ctx.enter_context(tc.tile_pool(name="consts", bufs=1))
        yrpool = ctx.enter_context(tc.tile_pool(name="yrpool", bufs=1))
        opool = ctx.enter_context(tc.tile_pool(name="opool", bufs=2))
        psD = ctx.enter_context(tc.tile_pool(name="psD", bufs=2, space="PSUM"))
        psE = ctx.enter_context(tc.tile_pool(name="psE", bufs=3, space="PSUM"))

        yb_ts = [consts.tile([128, 512], F32R, tag=f"yb{c}", name=f"yb{c}")
                 for c in range(4)]
        nc.sync.dma_start(out=yb_ts[0], in_=yb[:, 0:512])
        gh_t = consts.tile([128, NXH], F32R)
        nc.sync.dma_start(out=gh_t, in_=gh[:, :])
        cym_t = consts.tile([64, NY], F32R)
        nc.sync.dma_start(out=cym_t, in_=cym[:, :])
        for c in range(1, 4):
            nc.sync.dma_start(out=yb_ts[c], in_=yb[:, c * 512:(c + 1) * 512])

        # stage D: yr_i [m64, nx256] = yb[:, i]^T @ gh
        YRs = [yrpool.tile([64, 8 * NXH], F32R, tag=f"YR{gi}", bufs=1,
                           name=f"YR{gi}") for gi in range(4)]  # [m, (i%8, nx)]
        for ip in range(DV // 2):
            pD = psD.tile([64, 2 * NXH], F32)
            for ii in range(2):
                i = ip * 2 + ii
                nc.tensor.matmul(pD[:, ii * NXH:(ii + 1) * NXH],
                                 yb_ts[i // 8][:, (i % 8) * KY:
                                               (i % 8 + 1) * KY], gh_t,
                                 start=True, stop=True)
            i0 = ip * 2
            _copy(nc, ip, YRs[i0 // 8][:, (i0 % 8) * NXH:(i0 % 8 + 2) * NXH],
                  pD.bitcast(F32R))

        # stage E: out chunk [nx128, ny512] per (i, kc); assemble [nx, (ny, i)].
        # Two i's share one 2-bank psum tile so each drain copy writes
        # (ny, i-pair) with 8-byte contiguous runs instead of 4.
        HALF = NY // 2 * DV                      # 8192 cols per ny-half
        for kc in range(2):
            Oh_ts = [opool.tile([128, HALF], F32, tag=f"O{h}", bufs=2,
                                name=f"O{kc}_{h}") for h in range(2)]
            Ovs = [t.rearrange("p (ny i) -> p ny i", i=DV) for t in Oh_ts]
            for ip in range(DV // 2):
                pE = psE.tile([128, 2 * NY], F32)    # 2 banks
                for ii in range(2):
                    i = ip * 2 + ii
                    nc.tensor.matmul(pE[:, ii * NY:(ii + 1) * NY],
                                     YRs[i // 8][:, (i % 8) * NXH + kc * 128:
                                         (i % 8) * NXH + (kc + 1) * 128],
                                     cym_t, start=True, stop=True)
                pEv = pE.rearrange("p (i ny) -> p ny i", i=2)
                for h in range(2):
                    _copy(nc, ip + h, Ovs[h][:, :, ip * 2:ip * 2 + 2],
                          pEv[:, h * 256:(h + 1) * 256, :])
            for h in range(2):
                nc.sync.dma_start(
                    out=oh[kc * 128:(kc + 1) * 128, h * HALF:(h + 1) * HALF],
                    in_=Oh_ts[h])
    return _split_multiwait(nc)


_NC_CACHE = {}
LAST_EXEC_NS = []


def _get(name):
    if name not in _NC_CACHE:
        _NC_CACHE[name] = {"p1": build_phase1, "p2a": build_phase2a,
                           "p2b": build_phase2b}[name]()
    return _NC_CACHE[name]


def kernel(x, R_real, R_imag):
    x = np.ascontiguousarray(x, dtype=np.float32)
    AL = KX // NCORES

    # ---------------- phase 1 ----------------
    in1 = []
    for c in range(NCORES):
        b, h = c // 2, c % 2
        in1.append({
            "xh": x[b, h * NXH:(h + 1) * NXH].reshape(NXH, NY * DV),
            "fxt": _FXT[h * NXH:(h + 1) * NXH],
            "cyt": _CYT,
            "ident": _EYE,
        })
    LAST_EXEC_NS.clear()
    r1 = run_bass_kernel_spmd(_get("p1"), in1, list(range(NCORES)))
    LAST_EXEC_NS.append(r1.exec_time_ns)
    # partials [m, (j, alpha)] per (b, h)
    parts = [r1.results[c]["xtr"].reshape(KY, DV, 128) for c in range(NCORES)]
    xtr = np.stack([parts[2 * b] + parts[2 * b + 1] for b in range(B)])  # [B,KY,DV,128]

    # ---------------- phase 2a ----------------
    NMODE = AL * KY
    NG = NMODE // 2
    in2 = []
    for s in range(NCORES):
        a_sl = slice(s * AL, (s + 1) * AL)
        # [j, i, mode] slices of R
        Rr_t = R_real[:, :, a_sl, :].transpose(1, 0, 2, 3).reshape(DV, DV, NMODE)
        Ri_t = R_imag[:, :, a_sl, :].transpose(1, 0, 2, 3).reshape(DV, DV, NMODE)
        W2 = np.zeros((128, NG, 64), dtype=np.float32)
        xr = xtr[:, :, :, a_sl].transpose(2, 3, 1, 0).reshape(DV, NMODE, B)
        xi = (xtr[:, :, :, 64 + s * AL:64 + (s + 1) * AL]
              .transpose(2, 3, 1, 0).reshape(DV, NMODE, B))
        X2 = np.empty((128, NG, 2, B), dtype=np.float32)
        for u in range(2):
            r0, r1, r2_ = u * 64, u * 64 + 32, u * 64 + 64
            W2[r0:r1, :, u * 32:(u + 1) * 32] = (
                Rr_t[:, :, u::2].transpose(0, 2, 1))
            W2[r1:r2_, :, u * 32:(u + 1) * 32] = (
                Ri_t[:, :, u::2].transpose(0, 2, 1))
            X2[r0:r1, :, 0, :] = xr[:, u::2, :]
            X2[r1:r2_, :, 0, :] = -xi[:, u::2, :]
            X2[r0:r1, :, 1, :] = xi[:, u::2, :]
            X2[r1:r2_, :, 1, :] = xr[:, u::2, :]
        in2.append({"w2": W2.reshape(128, NG * 64),
                    "x2": X2.reshape(128, NG * 8)})
    r2 = run_bass_kernel_spmd(_get("p2a"), in2, list(range(NCORES)))
    LAST_EXEC_NS.append(r2.exec_time_ns)
    # y core result [64=(u,i), (g, q, b)] -> [q, i, a_l, m, b] per core
    ys = []
    for s in range(NCORES):
        t = r2.results[s]["y"].reshape(2, DV, NG, 2, B)       # [u, i, g, q, b]
        t = t.transpose(3, 1, 2, 0, 4).reshape(2, DV, NMODE, B)
        ys.append(t.reshape(2, DV, AL, KY, B))
    y = np.stack(ys)                                           # [s, q, i, a_l, m, b]
    y = y.transpose(1, 2, 0, 3, 4, 5).reshape(2, DV, KX, KY, B)  # [q, i, a, m, b]

    # ---------------- phase 2b ----------------
    in3 = []
    for c in range(NCORES):
        b, h = c // 2, c % 2
        # yb [(q, a), (i, m)]
        ybc = y[:, :, :, :, b].transpose(0, 2, 1, 3).reshape(128, DV * KY)
        in3.append({"yb": np.ascontiguousarray(ybc),
                    "gh": _G[:, h * NXH:(h + 1) * NXH],
                    "cym": _CY})
    r3 = run_bass_kernel_spmd(_get("p2b"), in3, list(range(NCORES)))
    LAST_EXEC_NS.append(r3.exec_time_ns)

    out = np.empty((B, NX, NY, DV), dtype=np.float32)
    for c in range(NCORES):
        b, h = c // 2, c % 2
        out[b, h * NXH:(h + 1) * NXH] = r3.results[c]["oh"].reshape(NXH, NY, DV)
    return out



# revision 5
# speedup vs baseline: 1.6350x; 1.6350x over previous
"""Trainium2 Bass kernel for the truncated-spectrum 2D conv (CF2DConv).

Math: out = iDCT_y( irfft_x( mix_per_mode( rfft_x( DCT_y(x) )[:64,:64] ) ) )
All transforms are dense truncated matrices; the whole op is a chain of
matmuls plus a per-mode complex channel mix.

v2: bf16 data path (fp32 PSUM accumulation), DCT-before-FFT ordering in the
forward pass (truncates Y 512->64 before the X transform, cutting PE work
~2x), dense PSUM drains in the inverse pass with host-side final transpose.

Execution: 3 SPMD launches on 8 NeuronCores.
  phase 1  (shard (b, nx-half)): partial forward spectrum per core
  phase 2a (shard a-modes):      per-mode complex mix, R read exactly once
  phase 2b (shard (b, nx-half)): inverse transforms, dense output writes
Host does the (cheap, few-MB) re-shards between launches.
"""
import numpy as np
import ml_dtypes
from contextlib import ExitStack

import concourse.bass as bass
import concourse.mybir as mybir
import concourse.tile as tile
from concourse.bass_utils import run_bass_kernel_spmd

B, NX, NY, DV = 4, 512, 512, 32
KX, KY = 64, 64
NCORES = 8
NXH = NX // 2          # 256 rows per (b, h) core
F32 = mybir.dt.float32
BF16 = mybir.dt.bfloat16
NPBF16 = ml_dtypes.bfloat16


def _split_multiwait(nc):
    """Each 64B engine instruction has ONE sync-wait slot; Tile can attach
    several (e.g. two operands arriving on different DMAHW sem lanes), which
    walrus codegen rejects ("Too many sync wait commands"). Spill excess
    waits (and updates) onto chains of single-wait no-ops on the same
    engine queue."""
    cnt = 0
    for fn in nc.m.functions:
        for blk in fn.blocks:
            insts = list(blk.instructions)
            out = []
            changed = False
            for inst in insts:
                si = inst.sync_info
                if si is not None:
                    waits = list(si.on_wait or [])
                    ups = list(si.on_update or [])
                    if len(waits) > 1:
                        for w in waits[:-1]:
                            cnt += 1
                            out.append(mybir.InstNoOp(
                                name=f"premw{cnt}_{inst.name}",
                                sync_info=mybir.SyncInfo(on_wait=[w],
                                                         on_update=[]),
                                bass_nofuse=True, engine=inst.engine))
                        inst.sync_info = mybir.SyncInfo(
                            on_wait=waits[-1:], on_update=ups)
                        changed = True
                    if len(ups) > 1:
                        inst.sync_info = mybir.SyncInfo(
                            on_wait=list(inst.sync_info.on_wait or []),
                            on_update=ups[:1])
                        out.append(inst)
                        for u in ups[1:]:
                            cnt += 1
                            out.append(mybir.InstNoOp(
                                name=f"postmw{cnt}_{inst.name}",
                                sync_info=mybir.SyncInfo(on_wait=[],
                                                         on_update=[u]),
                                bass_nofuse=True, engine=inst.engine))
                        changed = True
                        continue
                out.append(inst)
            if changed:
                blk.instructions = out
    return nc


# ----------------------------------------------------------------------------
# Host-side constant transform matrices (float64 -> bf16)
# ----------------------------------------------------------------------------


def _copy(nc, idx, out, in_):
    if idx % 2 == 0:
        nc.scalar.copy(out, in_)
    else:
        nc.vector.tensor_copy(out, in_)


def _build_consts():
    ny = np.arange(NY)
    m = np.arange(KY)
    Cy = np.cos(np.pi * (2 * ny[None, :] + 1) * m[:, None] / (2 * NY))
    s = np.full((KY, 1), np.sqrt(2.0 / NY)); s[0, 0] = np.sqrt(1.0 / NY)
    Cy = Cy * s                                     # [KY, NY]

    nx = np.arange(NX)
    a = np.arange(KX)
    ang = 2 * np.pi * a[:, None] * nx[None, :] / NX
    Fre = np.cos(ang) / np.sqrt(NX)                 # [KX, NX]
    Fim = -np.sin(ang) / np.sqrt(NX)

    w = np.full(KX, 2.0); w[0] = 1.0
    Gr = w[None, :] * np.cos(ang.T) / np.sqrt(NX)   # [NX, KX]
    Gi = -w[None, :] * np.sin(ang.T) / np.sqrt(NX)

    # FxT_all [NX, 128]: cols 0:64 = Fre^T, 64:128 = Fim^T
    FxT = np.concatenate([Fre.T, Fim.T], axis=1)    # [512, 128]
    CyT = Cy.T                                      # [512, 64] DCT lhsT
    # G_all [128(alpha), NX]: rows 0:64 = Gr^T, 64:128 = Gi^T
    G = np.concatenate([Gr.T, Gi.T], axis=0)        # [128, 512]
    return (FxT.astype(NPBF16), CyT.astype(NPBF16),
            G.astype(NPBF16), Cy.astype(NPBF16))


_FXT, _CYT, _G, _CY = _build_consts()
_EYE = np.eye(64, dtype=NPBF16)


# ----------------------------------------------------------------------------
# Phase 1: DCT-Y (contract ny, full) then rFFT-X (contract local nx half)
#   in : xt_in [512, 8192]  ny-major view of this core's x shard, bf16
#        cyt   [512, 64]    Cy^T (DCT lhsT)
#        fxt   [256, 128]   FxT rows for this nx-half
#        ident [64, 64]
#   out: xtr   [128, 2048]  [alpha, (j, m)] fp32  (partial: sum over h needed)
# ----------------------------------------------------------------------------
def build_phase1():
    nc = bass.Bass()
    xt_in = nc.declare_dram_parameter("xt_in", [NY, NXH * DV], BF16,
                                      isOutput=False)
    cyt = nc.declare_dram_parameter("cyt", [NY, KY], BF16, isOutput=False)
    fxt = nc.declare_dram_parameter("fxt", [NXH, 128], BF16, isOutput=False)
    ident = nc.declare_dram_parameter("ident", [64, 64], BF16, isOutput=False)
    xtr = nc.declare_dram_parameter("xtr", [128, DV * KY], F32, isOutput=True)

    with ExitStack() as ctx:
        tc = ctx.enter_context(tile.TileContext(nc))
        consts = ctx.enter_context(tc.tile_pool(name="consts", bufs=1))
        xpool = ctx.enter_context(tc.tile_pool(name="xpool", bufs=1))
        tpool = ctx.enter_context(tc.tile_pool(name="tpool", bufs=1))
        ttpool = ctx.enter_context(tc.tile_pool(name="ttpool", bufs=1))
        xtrpool = ctx.enter_context(tc.tile_pool(name="xtrpool", bufs=1))
        psD = ctx.enter_context(tc.tile_pool(name="psD", bufs=2, space="PSUM"))
        psT = ctx.enter_context(tc.tile_pool(name="psT", bufs=2, space="PSUM"))
        psF = ctx.enter_context(tc.tile_pool(name="psF", bufs=2, space="PSUM"))

        cyt_t = consts.tile([128, 4 * KY], BF16)     # chunk c at cols c*64
        for c in range(4):
            nc.sync.dma_start(out=cyt_t[:, c * KY:(c + 1) * KY],
                              in_=cyt[c * 128:(c + 1) * 128, :])
        fxt_t = consts.tile([128, 256], BF16)        # chunk c2 at cols c2*128
        for c2 in range(2):
            nc.sync.dma_start(out=fxt_t[:, c2 * 128:(c2 + 1) * 128],
                              in_=fxt[c2 * 128:(c2 + 1) * 128, :])
        id_t = consts.tile([64, 64], BF16)
        nc.sync.dma_start(out=id_t, in_=ident[:, :])

        # x tiles [128 ny, 4096]: (c = ny chunk, gh = col half); DMA in use
        # order (gh major) so the first DCT group can start after one tile.
        xts = {}
        for gh in range(2):
            for c in range(4):
                xt = xpool.tile([128, 4096], BF16, tag=f"xt{c}_{gh}",
                                name=f"xt{c}_{gh}")
                nc.sync.dma_start(
                    out=xt,
                    in_=xt_in[c * 128:(c + 1) * 128,
                              gh * 4096:(gh + 1) * 4096])
                xts[(c, gh)] = xt

        # ---- stage DCT-Y: T[m 64, (nx 256, j 32)] = Cy @ x ----
        T = tpool.tile([64, NXH * DV], BF16, tag="T", name="T")
        for g in range(8):                           # 1024-col groups
            gh, off = g // 4, (g % 4) * 1024
            ps = psD.tile([64, 1024], F32, tag="dct", name=f"dct{g}")
            for c in range(4):
                for s in range(2):
                    nc.tensor.matmul(
                        ps[:, s * 512:(s + 1) * 512],
                        cyt_t[:, c * KY:(c + 1) * KY],
                        xts[(c, gh)][:, off + s * 512:off + (s + 1) * 512],
                        start=(c == 0), stop=(c == 3))
            _copy(nc, g, T[:, g * 1024:(g + 1) * 1024], ps)

        # ---- transposes: TT[nx 128 (c2), (j 32, m 64)] = T^T per j ----
        Tv = T.rearrange("p (nx j) -> p nx j", j=DV)
        TT = ttpool.tile([128, 2 * DV * KY], BF16, tag="TT", name="TT")
        for c2 in range(2):
            for jg in range(4):
                pT = psT.tile([128, 512], BF16, tag="pT", name=f"pT{c2}_{jg}")
                for jj in range(8):
                    j = jg * 8 + jj
                    nc.tensor.transpose(
                        pT[:, jj * KY:(jj + 1) * KY],
                        Tv[:, c2 * 128:(c2 + 1) * 128, j], id_t)
                _copy(nc, jg, TT[:, c2 * 2048 + jg * 512:
                                 c2 * 2048 + (jg + 1) * 512], pT)

        # ---- stage rFFT-X (contract local nx): xtr[alpha, (j, m)] ----
        xtr_s = xtrpool.tile([128, DV * KY], F32, tag="xtr", name="xtr_s")
        for n in range(4):
            ps = psF.tile([128, 512], F32, tag="fft", name=f"fft{n}")
            for c2 in range(2):
                nc.tensor.matmul(
                    ps, fxt_t[:, c2 * 128:(c2 + 1) * 128],
                    TT[:, c2 * 2048 + n * 512:c2 * 2048 + (n + 1) * 512],
                    start=(c2 == 0), stop=(c2 == 1))
            _copy(nc, n, xtr_s[:, n * 512:(n + 1) * 512], ps)
            nc.sync.dma_start(out=xtr[:, n * 512:(n + 1) * 512],
                              in_=xtr_s[:, n * 512:(n + 1) * 512])
    return _split_multiwait(nc)


# ----------------------------------------------------------------------------
# Phase 2a: per-mode complex channel mix, sharded over a (8 a-values per core)
#   in : w2   [128, 256*64]  [(rr/ri, j), (g, i32)]  R slice, bf16
#        x2   [128, 256*8]   [(p, j), (g, q, b)] spectrum, bf16
#   out: y    [64, 8*64*4]   [(u, i), (g, q, b)] fp32
# ----------------------------------------------------------------------------
def build_phase2a():
    NMODE = (KX // NCORES) * KY                      # 512 modes per core
    NG = NMODE // 2                                  # 256 mode-pair groups
    nc = bass.Bass()
    w2 = nc.declare_dram_parameter("w2", [128, NG * 64], BF16, isOutput=False)
    x2 = nc.declare_dram_parameter("x2", [128, NG * 8], BF16, isOutput=False)
    y = nc.declare_dram_parameter("y", [64, NMODE * B], F32, isOutput=True)

    with ExitStack() as ctx:
        tc = ctx.enter_context(tile.TileContext(nc))
        consts = ctx.enter_context(tc.tile_pool(name="consts", bufs=1))
        outpool = ctx.enter_context(tc.tile_pool(name="outpool", bufs=1))
        psY = ctx.enter_context(tc.tile_pool(name="psY", bufs=4, space="PSUM"))

        x_ts = []
        for c in range(2):
            x_c = consts.tile([128, NG * 4], BF16, tag=f"x{c}", name=f"x{c}")
            nc.sync.dma_start(out=x_c, in_=x2[:, c * NG * 4:(c + 1) * NG * 4])
            x_ts.append(x_c)
        w_ts = []
        for c in range(8):
            w_c = consts.tile([128, 2048], BF16, tag=f"w{c}", name=f"w{c}")
            nc.sync.dma_start(out=w_c, in_=w2[:, c * 2048:(c + 1) * 2048])
            w_ts.append(w_c)
        y_ts = [outpool.tile([64, 512], F32, tag=f"y{bk}", name=f"y{bk}")
                for bk in range(4)]

        for bk in range(4):                          # 64 groups per psum bank
            pY = psY.tile([64, 512], F32)
            for gg in range(64):
                g = bk * 64 + gg
                nc.tensor.matmul(pY[:, gg * 8:(gg + 1) * 8],
                                 w_ts[g // 32][:, (g % 32) * 64:
                                               (g % 32 + 1) * 64],
                                 x_ts[g // 128][:, (g % 128) * 8:
                                                (g % 128 + 1) * 8],
                                 start=True, stop=True)
            _copy(nc, bk, y_ts[bk], pY)
            nc.sync.dma_start(out=y[:, bk * 512:(bk + 1) * 512], in_=y_ts[bk])
    return _split_multiwait(nc)


# ----------------------------------------------------------------------------
# Phase 2b: inverse transforms per (b, nx-half)
#   in : yb  [128, 2048]  [(q, a), (i, m)] bf16
#        gh  [128, 256]   G rows alpha, cols nx-local, bf16
#        cym [64, 512]    Cy [m, ny] bf16
#   out: oh2 [256, 16384] rows nx-local, cols (i, ny) bf16
# ----------------------------------------------------------------------------
def build_phase2b():
    nc = bass.Bass()
    yb = nc.declare_dram_parameter("yb", [128, DV * KY], BF16, isOutput=False)
    gh = nc.declare_dram_parameter("gh", [128, NXH], BF16, isOutput=False)
    cym = nc.declare_dram_parameter("cym", [KY, NY], BF16, isOutput=False)
    oh2 = nc.declare_dram_parameter("oh2", [NXH, DV * NY], BF16, isOutput=True)

    with ExitStack() as ctx:
        tc = ctx.enter_context(tile.TileContext(nc))
        consts = ctx.enter_context(tc.tile_pool(name="consts", bufs=1))
        yrpool = ctx.enter_context(tc.tile_pool(name="yrpool", bufs=1))
        opool = ctx.enter_context(tc.tile_pool(name="opool", bufs=2))
        psD = ctx.enter_context(tc.tile_pool(name="psD", bufs=2, space="PSUM"))
        psE = ctx.enter_context(tc.tile_pool(name="psE", bufs=4, space="PSUM"))

        yb_ts = [consts.tile([128, 512], BF16, tag=f"yb{c}", name=f"yb{c}")
                 for c in range(4)]
        nc.sync.dma_start(out=yb_ts[0], in_=yb[:, 0:512])
        gh_t = consts.tile([128, NXH], BF16)
        nc.sync.dma_start(out=gh_t, in_=gh[:, :])
        cym_t = consts.tile([64, NY], BF16)
        nc.sync.dma_start(out=cym_t, in_=cym[:, :])
        for c in range(1, 4):
            nc.sync.dma_start(out=yb_ts[c], in_=yb[:, c * 512:(c + 1) * 512])

        # stage D: yr_i [m64, nx256] = yb[:, i]^T @ gh
        YRs = [yrpool.tile([64, 8 * NXH], BF16, tag=f"YR{gi}", bufs=1,
                           name=f"YR{gi}") for gi in range(4)]  # [m, (i%8, nx)]
        for ip in range(DV // 2):
            pD = psD.tile([64, 2 * NXH], F32)
            for ii in range(2):
                i = ip * 2 + ii
                nc.tensor.matmul(pD[:, ii * NXH:(ii + 1) * NXH],
                                 yb_ts[i // 8][:, (i % 8) * KY:
                                               (i % 8 + 1) * KY], gh_t,
                                 start=True, stop=True)
            i0 = ip * 2
            _copy(nc, ip, YRs[i0 // 8][:, (i0 % 8) * NXH:(i0 % 8 + 2) * NXH],
                  pD)

        # stage E: out[nx 128, ny 512] per (i, kc); dense drains into
        # [nx, (i, ny)] tiles (host re-transposes to [nx, ny, i] for free).
        for kc in range(2):
            for ig in range(4):                      # 8 i's per output tile
                Oh = opool.tile([128, 8 * NY], BF16, tag="Oh",
                                name=f"Oh{kc}_{ig}")
                for ii in range(8):
                    i = ig * 8 + ii
                    pE = psE.tile([128, NY], F32)
                    nc.tensor.matmul(pE,
                                     YRs[i // 8][:, (i % 8) * NXH + kc * 128:
                                                 (i % 8) * NXH + (kc + 1) * 128],
                                     cym_t, start=True, stop=True)
                    _copy(nc, i, Oh[:, ii * NY:(ii + 1) * NY], pE)
                nc.sync.dma_start(
                    out=oh2[kc * 128:(kc + 1) * 128,
                            ig * 8 * NY:(ig + 1) * 8 * NY],
                    in_=Oh)
    return _split_multiwait(nc)


_NC_CACHE = {}
LAST_EXEC_NS = []


def _get(name):
    if name not in _NC_CACHE:
        _NC_CACHE[name] = {"p1": build_phase1, "p2a": build_phase2a,
                           "p2b": build_phase2b}[name]()
    return _NC_CACHE[name]


def kernel(x, R_real, R_imag):
    x = np.ascontiguousarray(x, dtype=np.float32)
    AL = KX // NCORES

    # ---------------- phase 1 ----------------
    in1 = []
    for c in range(NCORES):
        b, h = c // 2, c % 2
        xh = x[b, h * NXH:(h + 1) * NXH]              # [256, 512, 32]
        xt = np.ascontiguousarray(xh.transpose(1, 0, 2)).astype(NPBF16)
        in1.append({
            "xt_in": xt.reshape(NY, NXH * DV),
            "cyt": _CYT,
            "fxt": _FXT[h * NXH:(h + 1) * NXH],
            "ident": _EYE,
        })
    LAST_EXEC_NS.clear()
    r1 = run_bass_kernel_spmd(_get("p1"), in1, list(range(NCORES)))
    LAST_EXEC_NS.append(r1.exec_time_ns)
    # partials [alpha, j, m] per (b, h); sum halves -> spect [B, 128, 32, 64]
    parts = [r1.results[c]["xtr"].reshape(128, DV, KY) for c in range(NCORES)]
    spect = np.stack([parts[2 * b] + parts[2 * b + 1] for b in range(B)])

    # ---------------- phase 2a ----------------
    NMODE = AL * KY
    NG = NMODE // 2
    in2 = []
    for s in range(NCORES):
        a_sl = slice(s * AL, (s + 1) * AL)
        # [j, i, mode] slices of R (mode = a_l*64 + m)
        Rr_t = R_real[:, :, a_sl, :].transpose(1, 0, 2, 3).reshape(DV, DV, NMODE)
        Ri_t = R_imag[:, :, a_sl, :].transpose(1, 0, 2, 3).reshape(DV, DV, NMODE)
        W2 = np.zeros((128, NG, 64), dtype=np.float32)
        # spect [B, alpha, j, m] -> xr/xi [j, mode, b]
        xr = spect[:, a_sl, :, :].transpose(2, 1, 3, 0).reshape(DV, NMODE, B)
        xi = (spect[:, 64 + s * AL:64 + (s + 1) * AL, :, :]
              .transpose(2, 1, 3, 0).reshape(DV, NMODE, B))
        X2 = np.empty((128, NG, 2, B), dtype=np.float32)
        for u in range(2):
            r0, r1_, r2_ = u * 64, u * 64 + 32, u * 64 + 64
            W2[r0:r1_, :, u * 32:(u + 1) * 32] = (
                Rr_t[:, :, u::2].transpose(0, 2, 1))
            W2[r1_:r2_, :, u * 32:(u + 1) * 32] = (
                Ri_t[:, :, u::2].transpose(0, 2, 1))
            X2[r0:r1_, :, 0, :] = xr[:, u::2, :]
            X2[r1_:r2_, :, 0, :] = -xi[:, u::2, :]
            X2[r0:r1_, :, 1, :] = xi[:, u::2, :]
            X2[r1_:r2_, :, 1, :] = xr[:, u::2, :]
        in2.append({"w2": W2.reshape(128, NG * 64).astype(NPBF16),
                    "x2": X2.reshape(128, NG * 8).astype(NPBF16)})
    r2 = run_bass_kernel_spmd(_get("p2a"), in2, list(range(NCORES)))
    LAST_EXEC_NS.append(r2.exec_time_ns)
    # y core result [64=(u,i), (g, q, b)] -> [q, i, a_l, m, b] per core
    ys = []
    for s in range(NCORES):
        t = r2.results[s]["y"].reshape(2, DV, NG, 2, B)       # [u, i, g, q, b]
        t = t.transpose(3, 1, 2, 0, 4).reshape(2, DV, NMODE, B)
        ys.append(t.reshape(2, DV, AL, KY, B))
    yfull = np.stack(ys)                                       # [s, q, i, a_l, m, b]
    yfull = yfull.transpose(1, 2, 0, 3, 4, 5).reshape(2, DV, KX, KY, B)

    # ---------------- phase 2b ----------------
    in3 = []
    for c in range(NCORES):
        b, h = c // 2, c % 2
        # yb [(q, a), (i, m)]
        ybc = yfull[:, :, :, :, b].transpose(0, 2, 1, 3).reshape(128, DV * KY)
        in3.append({"yb": np.ascontiguousarray(ybc).astype(NPBF16),
                    "gh": np.ascontiguousarray(_G[:, h * NXH:(h + 1) * NXH]),
                    "cym": _CY})
    r3 = run_bass_kernel_spmd(_get("p2b"), in3, list(range(NCORES)))
    LAST_EXEC_NS.append(r3.exec_time_ns)

    out = np.empty((B, NX, NY, DV), dtype=np.float32)
    for c in range(NCORES):
        b, h = c // 2, c % 2
        oh2 = r3.results[c]["oh2"].reshape(NXH, DV, NY)
        out[b, h * NXH:(h + 1) * NXH] = (
            oh2.transpose(0, 2, 1).astype(np.float32))
    return out


# revision 15
# speedup vs baseline: 1.6906x; 1.0340x over previous
"""Trainium2 Bass kernel for the truncated-spectrum 2D conv (CF2DConv).

Math: out = iDCT_y( irfft_x( mix_per_mode( rfft_x( DCT_y(x) )[:64,:64] ) ) )
All transforms are dense truncated matrices; the whole op is a chain of
matmuls plus a per-mode complex channel mix.

v2: bf16 data path (fp32 PSUM accumulation), DCT-before-FFT ordering in the
forward pass (truncates Y 512->64 before the X transform, cutting PE work
~2x), dense PSUM drains in the inverse pass with host-side final transpose.

Execution: 3 SPMD launches on 8 NeuronCores.
  phase 1  (shard (b, nx-half)): partial forward spectrum per core
  phase 2a (shard a-modes):      per-mode complex mix, R read exactly once
  phase 2b (shard (b, nx-half)): inverse transforms, dense output writes
Host does the (cheap, few-MB) re-shards between launches.
"""
import numpy as np
import ml_dtypes
from contextlib import ExitStack

import concourse.bass as bass
import concourse.mybir as mybir
import concourse.tile as tile
from concourse.bass_utils import run_bass_kernel_spmd

B, NX, NY, DV = 4, 512, 512, 32
KX, KY = 64, 64
NCORES = 8
NXH = NX // 2          # 256 rows per (b, h) core
F32 = mybir.dt.float32
BF16 = mybir.dt.bfloat16
NPBF16 = ml_dtypes.bfloat16


def _split_multiwait(nc):
    """Each 64B engine instruction has ONE sync-wait slot; Tile can attach
    several (e.g. two operands arriving on different DMAHW sem lanes), which
    walrus codegen rejects ("Too many sync wait commands"). Spill excess
    waits (and updates) onto chains of single-wait no-ops on the same
    engine queue."""
    cnt = 0
    for fn in nc.m.functions:
        for blk in fn.blocks:
            insts = list(blk.instructions)
            out = []
            changed = False
            for inst in insts:
                si = inst.sync_info
                if si is not None:
                    waits = list(si.on_wait or [])
                    ups = list(si.on_update or [])
                    if len(waits) > 1:
                        for w in waits[:-1]:
                            cnt += 1
                            out.append(mybir.InstNoOp(
                                name=f"premw{cnt}_{inst.name}",
                                sync_info=mybir.SyncInfo(on_wait=[w],
                                                         on_update=[]),
                                bass_nofuse=True, engine=inst.engine))
                        inst.sync_info = mybir.SyncInfo(
                            on_wait=waits[-1:], on_update=ups)
                        changed = True
                    if len(ups) > 1:
                        inst.sync_info = mybir.SyncInfo(
                            on_wait=list(inst.sync_info.on_wait or []),
                            on_update=ups[:1])
                        out.append(inst)
                        for u in ups[1:]:
                            cnt += 1
                            out.append(mybir.InstNoOp(
                                name=f"postmw{cnt}_{inst.name}",
                                sync_info=mybir.SyncInfo(on_wait=[],
                                                         on_update=[u]),
                                bass_nofuse=True, engine=inst.engine))
                        changed = True
                        continue
                out.append(inst)
            if changed:
                blk.instructions = out
    return nc


# ----------------------------------------------------------------------------
# Host-side constant transform matrices (float64 -> bf16)
# ----------------------------------------------------------------------------


def _copy(nc, idx, out, in_):
    if idx % 2 == 0:
        nc.scalar.copy(out, in_)
    else:
        nc.vector.tensor_copy(out, in_)


def _copy3(nc, idx, out, in_):
    r = idx % 3
    if r == 0:
        nc.scalar.copy(out, in_)
    elif r == 1:
        nc.vector.tensor_copy(out, in_)
    else:
        nc.gpsimd.tensor_copy(out, in_)


def _build_consts():
    ny = np.arange(NY)
    m = np.arange(KY)
    Cy = np.cos(np.pi * (2 * ny[None, :] + 1) * m[:, None] / (2 * NY))
    s = np.full((KY, 1), np.sqrt(2.0 / NY)); s[0, 0] = np.sqrt(1.0 / NY)
    Cy = Cy * s                                     # [KY, NY]

    nx = np.arange(NX)
    a = np.arange(KX)
    ang = 2 * np.pi * a[:, None] * nx[None, :] / NX
    Fre = np.cos(ang) / np.sqrt(NX)                 # [KX, NX]
    Fim = -np.sin(ang) / np.sqrt(NX)

    w = np.full(KX, 2.0); w[0] = 1.0
    Gr = w[None, :] * np.cos(ang.T) / np.sqrt(NX)   # [NX, KX]
    Gi = -w[None, :] * np.sin(ang.T) / np.sqrt(NX)

    # FxT_all [NX, 128]: cols 0:64 = Fre^T, 64:128 = Fim^T
    FxT = np.concatenate([Fre.T, Fim.T], axis=1)    # [512, 128]
    CyT = Cy.T                                      # [512, 64] DCT lhsT
    # G_all [128(alpha), NX]: rows 0:64 = Gr^T, 64:128 = Gi^T
    G = np.concatenate([Gr.T, Gi.T], axis=0)        # [128, 512]
    return (FxT.astype(NPBF16), CyT.astype(NPBF16),
            G.astype(NPBF16), Cy.astype(NPBF16))


_FXT, _CYT, _G, _CY = _build_consts()
_EYE = np.eye(64, dtype=NPBF16)


def _pack_phase1_consts(h):
    cpk = np.zeros((128, 576), dtype=NPBF16)
    for c in range(4):
        cpk[:, c * 64:(c + 1) * 64] = _CYT[c * 128:(c + 1) * 128]
    fxt = _FXT[h * NXH:(h + 1) * NXH]
    for c2 in range(2):
        cpk[:, 256 + c2 * 128:256 + (c2 + 1) * 128] = (
            fxt[c2 * 128:(c2 + 1) * 128])
    cpk[0:64, 512:576] = _EYE
    return cpk


_CPK = [_pack_phase1_consts(0), _pack_phase1_consts(1)]


# ----------------------------------------------------------------------------
# Phase 1: DCT-Y (contract ny, full) then rFFT-X (contract local nx half)
#   in : xt_in [512, 8192]  ny-major view of this core's x shard, bf16
#        cyt   [512, 64]    Cy^T (DCT lhsT)
#        fxt   [256, 128]   FxT rows for this nx-half
#        ident [64, 64]
#   out: xtr   [128, 2048]  [alpha, (j, m)] fp32  (partial: sum over h needed)
# ----------------------------------------------------------------------------
def build_phase1():
    nc = bass.Bass()
    xt_in = nc.declare_dram_parameter("xt_in", [NY, NXH * DV], BF16,
                                      isOutput=False)
    # packed consts: cols 0:256 cyt (c at c*64), 256:512 fxt (c2 at c2*128),
    # 512:576 identity (rows 0:64)
    cpk = nc.declare_dram_parameter("cpk", [128, 576], BF16, isOutput=False)
    xtr = nc.declare_dram_parameter("xtr", [128, DV * KY], F32, isOutput=True)

    with ExitStack() as ctx:
        tc = ctx.enter_context(tile.TileContext(nc))
        consts = ctx.enter_context(tc.tile_pool(name="consts", bufs=1))
        xpool = ctx.enter_context(tc.tile_pool(name="xpool", bufs=1))
        tpool = ctx.enter_context(tc.tile_pool(name="tpool", bufs=1))
        ttpool = ctx.enter_context(tc.tile_pool(name="ttpool", bufs=1))
        xtrpool = ctx.enter_context(tc.tile_pool(name="xtrpool", bufs=1))
        psD = ctx.enter_context(tc.tile_pool(name="psD", bufs=2, space="PSUM"))
        psT = ctx.enter_context(tc.tile_pool(name="psT", bufs=2, space="PSUM"))
        psF = ctx.enter_context(tc.tile_pool(name="psF", bufs=2, space="PSUM"))

        cpk_t = consts.tile([128, 576], BF16)
        nc.sync.dma_start(out=cpk_t, in_=cpk[:, :])
        id_t = cpk_t[0:64, 512:576]

        # x tiles [128 ny, 4096]: (c = ny chunk, gh = col half); DMA in use
        # order (gh major) so the first DCT group can start after one tile.
        xts = {}
        for gh in range(2):
            for c in range(4):
                xt = xpool.tile([128, 4096], BF16, tag=f"xt{c}_{gh}",
                                name=f"xt{c}_{gh}")
                nc.sync.dma_start(
                    out=xt,
                    in_=xt_in[c * 128:(c + 1) * 128,
                              gh * 4096:(gh + 1) * 4096])
                xts[(c, gh)] = xt

        # ---- stage DCT-Y: T[m 64, (nx 256, j 32)] = Cy @ x ----
        T = tpool.tile([64, NXH * DV], BF16, tag="T", name="T")
        for g in range(8):                           # 1024-col groups
            gh, off = g // 4, (g % 4) * 1024
            ps = psD.tile([64, 1024], F32, tag="dct", name=f"dct{g}")
            for c in range(4):
                for s in range(2):
                    nc.tensor.matmul(
                        ps[:, s * 512:(s + 1) * 512],
                        cpk_t[:, c * KY:(c + 1) * KY],
                        xts[(c, gh)][:, off + s * 512:off + (s + 1) * 512],
                        start=(c == 0), stop=(c == 3))
            _copy(nc, g, T[:, g * 1024:(g + 1) * 1024], ps)

        # ---- transposes: TT[nx 128 (c2), (j 32, m 64)] = T^T per j ----
        Tv = T.rearrange("p (nx j) -> p nx j", j=DV)
        TT = ttpool.tile([128, 2 * DV * KY], BF16, tag="TT", name="TT")
        for c2 in range(2):
            for jg in range(4):
                pT = psT.tile([128, 512], BF16, tag="pT", name=f"pT{c2}_{jg}")
                for jj in range(8):
                    j = jg * 8 + jj
                    nc.tensor.transpose(
                        pT[:, jj * KY:(jj + 1) * KY],
                        Tv[:, c2 * 128:(c2 + 1) * 128, j], id_t)
                _copy(nc, jg, TT[:, c2 * 2048 + jg * 512:
                                 c2 * 2048 + (jg + 1) * 512], pT)

        # ---- stage rFFT-X (contract local nx): xtr[alpha, (j, m)] ----
        xtr_s = xtrpool.tile([128, DV * KY], F32, tag="xtr", name="xtr_s")
        for n in range(4):
            ps = psF.tile([128, 512], F32, tag="fft", name=f"fft{n}")
            for c2 in range(2):
                nc.tensor.matmul(
                    ps, cpk_t[:, 256 + c2 * 128:256 + (c2 + 1) * 128],
                    TT[:, c2 * 2048 + n * 512:c2 * 2048 + (n + 1) * 512],
                    start=(c2 == 0), stop=(c2 == 1))
            _copy(nc, n, xtr_s[:, n * 512:(n + 1) * 512], ps)
            nc.sync.dma_start(out=xtr[:, n * 512:(n + 1) * 512],
                              in_=xtr_s[:, n * 512:(n + 1) * 512])
    return _split_multiwait(nc)


# ----------------------------------------------------------------------------
# Phase 2a: per-mode complex channel mix, sharded over a (8 a-values per core)
#   in : w2   [128, 256*64]  [(rr/ri, j), (g, i32)]  R slice, bf16
#        x2   [128, 256*8]   [(p, j), (g, q, b)] spectrum, bf16
#   out: y    [64, 8*64*4]   [(u, i), (g, q, b)] fp32
# ----------------------------------------------------------------------------
def build_phase2a():
    NMODE = (KX // NCORES) * KY                      # 512 modes per core
    NG = NMODE // 2                                  # 256 mode-pair groups
    nc = bass.Bass()
    w2 = nc.declare_dram_parameter("w2", [128, NG * 64], BF16, isOutput=False)
    x2 = nc.declare_dram_parameter("x2", [128, NG * 8], BF16, isOutput=False)
    y = nc.declare_dram_parameter("y", [64, NMODE * B], F32, isOutput=True)

    with ExitStack() as ctx:
        tc = ctx.enter_context(tile.TileContext(nc))
        consts = ctx.enter_context(tc.tile_pool(name="consts", bufs=1))
        outpool = ctx.enter_context(tc.tile_pool(name="outpool", bufs=1))
        psY = ctx.enter_context(tc.tile_pool(name="psY", bufs=4, space="PSUM"))

        x_ts = []
        for c in range(2):
            x_c = consts.tile([128, NG * 4], BF16, tag=f"x{c}", name=f"x{c}")
            nc.sync.dma_start(out=x_c, in_=x2[:, c * NG * 4:(c + 1) * NG * 4])
            x_ts.append(x_c)
        w_ts = []
        for c in range(8):
            w_c = consts.tile([128, 2048], BF16, tag=f"w{c}", name=f"w{c}")
            nc.sync.dma_start(out=w_c, in_=w2[:, c * 2048:(c + 1) * 2048])
            w_ts.append(w_c)
        y_ts = [outpool.tile([64, 512], F32, tag=f"y{bk}", name=f"y{bk}")
                for bk in range(4)]

        for bk in range(4):                          # 64 groups per psum bank
            pY = psY.tile([64, 512], F32)
            for gg in range(64):
                g = bk * 64 + gg
                nc.tensor.matmul(pY[:, gg * 8:(gg + 1) * 8],
                                 w_ts[g // 32][:, (g % 32) * 64:
                                               (g % 32 + 1) * 64],
                                 x_ts[g // 128][:, (g % 128) * 8:
                                                (g % 128 + 1) * 8],
                                 start=True, stop=True)
            _copy(nc, bk, y_ts[bk], pY)
            nc.sync.dma_start(out=y[:, bk * 512:(bk + 1) * 512], in_=y_ts[bk])
    return _split_multiwait(nc)


# ----------------------------------------------------------------------------
# Phase 2b: inverse transforms per (b, nx-half)
#   in : yb  [128, 2048]  [(q, a), (i, m)] bf16
#        gh  [128, 256]   G rows alpha, cols nx-local, bf16
#        cym [64, 512]    Cy [m, ny] bf16
#   out: oh2 [256, 16384] rows nx-local, cols (i, ny) bf16
# ----------------------------------------------------------------------------
def build_phase2b():
    nc = bass.Bass()
    # packed: cols 0:2048 yb [(q,a),(i,m)], 2048:2304 gh, 2304:2816 cym(r0:64)
    ypk = nc.declare_dram_parameter("ypk", [128, 2816], BF16, isOutput=False)
    oh2 = nc.declare_dram_parameter("oh2", [NXH, DV * NY], BF16, isOutput=True)

    with ExitStack() as ctx:
        tc = ctx.enter_context(tile.TileContext(nc))
        consts = ctx.enter_context(tc.tile_pool(name="consts", bufs=1))
        yrpool = ctx.enter_context(tc.tile_pool(name="yrpool", bufs=1))
        opool = ctx.enter_context(tc.tile_pool(name="opool", bufs=2))
        psD = ctx.enter_context(tc.tile_pool(name="psD", bufs=2, space="PSUM"))
        psE = ctx.enter_context(tc.tile_pool(name="psE", bufs=6, space="PSUM"))

        ypk_t = consts.tile([128, 2816], BF16)
        nc.sync.dma_start(out=ypk_t, in_=ypk[:, :])
        gh_t = ypk_t[:, 2048:2304]
        cym_t = ypk_t[0:64, 2304:2816]

        # stage D: yr_i [m64, nx256] = yb[:, i]^T @ gh
        YRs = [yrpool.tile([64, 2 * NXH], BF16, tag=f"YR{ip}", bufs=1,
                           name=f"YR{ip}") for ip in range(16)]  # [m,(i2,nx)]
        for ip in range(DV // 2):
            pD = psD.tile([64, 2 * NXH], F32)
            for ii in range(2):
                i = ip * 2 + ii
                nc.tensor.matmul(pD[:, ii * NXH:(ii + 1) * NXH],
                                 ypk_t[:, i * KY:(i + 1) * KY], gh_t,
                                 start=True, stop=True)
            _copy(nc, ip, YRs[ip], pD)

        # stage E: out[nx 128, ny 512] per (i, kc); dense drains into
        # [nx, (i, ny)] tiles (host re-transposes to [nx, ny, i] for free).
        for kc in range(2):
            for ig in range(4):                      # 8 i's per output tile
                Oh = opool.tile([128, 8 * NY], BF16, tag="Oh",
                                name=f"Oh{kc}_{ig}")
                for ii in range(8):
                    i = ig * 8 + ii
                    pE = psE.tile([128, NY], F32)
                    nc.tensor.matmul(pE,
                                     YRs[i // 2][:, (i % 2) * NXH + kc * 128:
                                                 (i % 2) * NXH + (kc + 1) * 128],
                                     cym_t, start=True, stop=True)
                    _copy(nc, i, Oh[:, ii * NY:(ii + 1) * NY], pE)
                nc.sync.dma_start(
                    out=oh2[kc * 128:(kc + 1) * 128,
                            ig * 8 * NY:(ig + 1) * 8 * NY],
                    in_=Oh)
    return _split_multiwait(nc)


_NC_CACHE = {}
LAST_EXEC_NS = []


def _get(name):
    if name not in _NC_CACHE:
        _NC_CACHE[name] = {"p1": build_phase1, "p2a": build_phase2a,
                           "p2b": build_phase2b}[name]()
    return _NC_CACHE[name]


def kernel(x, R_real, R_imag):
    x = np.ascontiguousarray(x, dtype=np.float32)
    AL = KX // NCORES

    # ---------------- phase 1 ----------------
    in1 = []
    for c in range(NCORES):
        b, h = c // 2, c % 2
        xh = x[b, h * NXH:(h + 1) * NXH]              # [256, 512, 32]
        xt = np.ascontiguousarray(xh.transpose(1, 0, 2)).astype(NPBF16)
        in1.append({
            "xt_in": xt.reshape(NY, NXH * DV),
            "cpk": _CPK[h],
        })
    LAST_EXEC_NS.clear()
    r1 = run_bass_kernel_spmd(_get("p1"), in1, list(range(NCORES)))
    LAST_EXEC_NS.append(r1.exec_time_ns)
    # partials [alpha, j, m] per (b, h); sum halves -> spect [B, 128, 32, 64]
    parts = [r1.results[c]["xtr"].reshape(128, DV, KY) for c in range(NCORES)]
    spect = np.stack([parts[2 * b] + parts[2 * b + 1] for b in range(B)])

    # ---------------- phase 2a ----------------
    NMODE = AL * KY
    NG = NMODE // 2
    in2 = []
    for s in range(NCORES):
        a_sl = slice(s * AL, (s + 1) * AL)
        # [j, i, mode] slices of R (mode = a_l*64 + m)
        Rr_t = R_real[:, :, a_sl, :].transpose(1, 0, 2, 3).reshape(DV, DV, NMODE)
        Ri_t = R_imag[:, :, a_sl, :].transpose(1, 0, 2, 3).reshape(DV, DV, NMODE)
        W2 = np.zeros((128, NG, 64), dtype=np.float32)
        # spect [B, alpha, j, m] -> xr/xi [j, mode, b]
        xr = spect[:, a_sl, :, :].transpose(2, 1, 3, 0).reshape(DV, NMODE, B)
        xi = (spect[:, 64 + s * AL:64 + (s + 1) * AL, :, :]
              .transpose(2, 1, 3, 0).reshape(DV, NMODE, B))
        X2 = np.empty((128, NG, 2, B), dtype=np.float32)
        for u in range(2):
            r0, r1_, r2_ = u * 64, u * 64 + 32, u * 64 + 64
            W2[r0:r1_, :, u * 32:(u + 1) * 32] = (
                Rr_t[:, :, u::2].transpose(0, 2, 1))
            W2[r1_:r2_, :, u * 32:(u + 1) * 32] = (
                Ri_t[:, :, u::2].transpose(0, 2, 1))
            X2[r0:r1_, :, 0, :] = xr[:, u::2, :]
            X2[r1_:r2_, :, 0, :] = -xi[:, u::2, :]
            X2[r0:r1_, :, 1, :] = xi[:, u::2, :]
            X2[r1_:r2_, :, 1, :] = xr[:, u::2, :]
        in2.append({"w2": W2.reshape(128, NG * 64).astype(NPBF16),
                    "x2": X2.reshape(128, NG * 8).astype(NPBF16)})
    r2 = run_bass_kernel_spmd(_get("p2a"), in2, list(range(NCORES)))
    LAST_EXEC_NS.append(r2.exec_time_ns)
    # y core result [64=(u,i), (g, q, b)] -> [q, i, a_l, m, b] per core
    ys = []
    for s in range(NCORES):
        t = r2.results[s]["y"].reshape(2, DV, NG, 2, B)       # [u, i, g, q, b]
        t = t.transpose(3, 1, 2, 0, 4).reshape(2, DV, NMODE, B)
        ys.append(t.reshape(2, DV, AL, KY, B))
    yfull = np.stack(ys)                                       # [s, q, i, a_l, m, b]
    yfull = yfull.transpose(1, 2, 0, 3, 4, 5).reshape(2, DV, KX, KY, B)

    # ---------------- phase 2b ----------------
    in3 = []
    for c in range(NCORES):
        b, h = c // 2, c % 2
        ypk = np.zeros((128, 2816), dtype=NPBF16)
        # yb [(q, a), (i, m)]
        ybc = yfull[:, :, :, :, b].transpose(0, 2, 1, 3).reshape(128, DV * KY)
        ypk[:, 0:2048] = ybc.astype(NPBF16)
        ypk[:, 2048:2304] = _G[:, h * NXH:(h + 1) * NXH]
        ypk[0:64, 2304:2816] = _CY
        in3.append({"ypk": ypk})
    r3 = run_bass_kernel_spmd(_get("p2b"), in3, list(range(NCORES)))
    LAST_EXEC_NS.append(r3.exec_time_ns)

    out = np.empty((B, NX, NY, DV), dtype=np.float32)
    for c in range(NCORES):
        b, h = c // 2, c % 2
        oh2 = r3.results[c]["oh2"].reshape(NXH, DV, NY)
        out[b, h * NXH:(h + 1) * NXH] = (
            oh2.transpose(0, 2, 1).astype(np.float32))
    return out


# revision 17
# speedup vs baseline: 1.7312x; 1.0240x over previous
"""Trainium2 Bass kernel for the truncated-spectrum 2D conv (CF2DConv).

Math: out = iDCT_y( irfft_x( mix_per_mode( rfft_x( DCT_y(x) )[:64,:64] ) ) )
All transforms are dense truncated matrices; the whole op is a chain of
matmuls plus a per-mode complex channel mix.

v2: bf16 data path (fp32 PSUM accumulation), DCT-before-FFT ordering in the
forward pass (truncates Y 512->64 before the X transform, cutting PE work
~2x), dense PSUM drains in the inverse pass with host-side final transpose.

Execution: 3 SPMD launches on 8 NeuronCores.
  phase 1  (shard (b, nx-half)): partial forward spectrum per core
  phase 2a (shard a-modes):      per-mode complex mix, R read exactly once
  phase 2b (shard (b, nx-half)): inverse transforms, dense output writes
Host does the (cheap, few-MB) re-shards between launches.
"""
import numpy as np
import ml_dtypes
from contextlib import ExitStack

import concourse.bass as bass
import concourse.mybir as mybir
import concourse.tile as tile
from concourse.bass_utils import run_bass_kernel_spmd

B, NX, NY, DV = 4, 512, 512, 32
KX, KY = 64, 64
NCORES = 8
NXH = NX // 2          # 256 rows per (b, h) core
F32 = mybir.dt.float32
BF16 = mybir.dt.bfloat16
NPBF16 = ml_dtypes.bfloat16


def _split_multiwait(nc):
    """Each 64B engine instruction has ONE sync-wait slot; Tile can attach
    several (e.g. two operands arriving on different DMAHW sem lanes), which
    walrus codegen rejects ("Too many sync wait commands"). Spill excess
    waits (and updates) onto chains of single-wait no-ops on the same
    engine queue."""
    cnt = 0
    for fn in nc.m.functions:
        for blk in fn.blocks:
            insts = list(blk.instructions)
            out = []
            changed = False
            for inst in insts:
                si = inst.sync_info
                if si is not None:
                    waits = list(si.on_wait or [])
                    ups = list(si.on_update or [])
                    if len(waits) > 1:
                        for w in waits[:-1]:
                            cnt += 1
                            out.append(mybir.InstNoOp(
                                name=f"premw{cnt}_{inst.name}",
                                sync_info=mybir.SyncInfo(on_wait=[w],
                                                         on_update=[]),
                                bass_nofuse=True, engine=inst.engine))
                        inst.sync_info = mybir.SyncInfo(
                            on_wait=waits[-1:], on_update=ups)
                        changed = True
                    if len(ups) > 1:
                        inst.sync_info = mybir.SyncInfo(
                            on_wait=list(inst.sync_info.on_wait or []),
                            on_update=ups[:1])
                        out.append(inst)
                        for u in ups[1:]:
                            cnt += 1
                            out.append(mybir.InstNoOp(
                                name=f"postmw{cnt}_{inst.name}",
                                sync_info=mybir.SyncInfo(on_wait=[],
                                                         on_update=[u]),
                                bass_nofuse=True, engine=inst.engine))
                        changed = True
                        continue
                out.append(inst)
            if changed:
                blk.instructions = out
    return nc


# ----------------------------------------------------------------------------
# Host-side constant transform matrices (float64 -> bf16)
# ----------------------------------------------------------------------------


def _copy(nc, idx, out, in_):
    if idx % 2 == 0:
        nc.scalar.copy(out, in_)
    else:
        nc.vector.tensor_copy(out, in_)


def _copy3(nc, idx, out, in_):
    r = idx % 3
    if r == 0:
        nc.scalar.copy(out, in_)
    elif r == 1:
        nc.vector.tensor_copy(out, in_)
    else:
        nc.gpsimd.tensor_copy(out, in_)


def _build_consts():
    ny = np.arange(NY)
    m = np.arange(KY)
    Cy = np.cos(np.pi * (2 * ny[None, :] + 1) * m[:, None] / (2 * NY))
    s = np.full((KY, 1), np.sqrt(2.0 / NY)); s[0, 0] = np.sqrt(1.0 / NY)
    Cy = Cy * s                                     # [KY, NY]

    nx = np.arange(NX)
    a = np.arange(KX)
    ang = 2 * np.pi * a[:, None] * nx[None, :] / NX
    Fre = np.cos(ang) / np.sqrt(NX)                 # [KX, NX]
    Fim = -np.sin(ang) / np.sqrt(NX)

    w = np.full(KX, 2.0); w[0] = 1.0
    Gr = w[None, :] * np.cos(ang.T) / np.sqrt(NX)   # [NX, KX]
    Gi = -w[None, :] * np.sin(ang.T) / np.sqrt(NX)

    # FxT_all [NX, 128]: cols 0:64 = Fre^T, 64:128 = Fim^T
    FxT = np.concatenate([Fre.T, Fim.T], axis=1)    # [512, 128]
    CyT = Cy.T                                      # [512, 64] DCT lhsT
    # G_all [128(alpha), NX]: rows 0:64 = Gr^T, 64:128 = Gi^T
    G = np.concatenate([Gr.T, Gi.T], axis=0)        # [128, 512]
    return (FxT.astype(NPBF16), CyT.astype(NPBF16),
            G.astype(NPBF16), Cy.astype(NPBF16))


_FXT, _CYT, _G, _CY = _build_consts()
_EYE = np.eye(64, dtype=NPBF16)


def _pack_phase1_consts(h):
    cpk = np.zeros((128, 576), dtype=NPBF16)
    for c in range(4):
        cpk[:, c * 64:(c + 1) * 64] = _CYT[c * 128:(c + 1) * 128]
    fxt = _FXT[h * NXH:(h + 1) * NXH]
    for c2 in range(2):
        cpk[:, 256 + c2 * 128:256 + (c2 + 1) * 128] = (
            fxt[c2 * 128:(c2 + 1) * 128])
    cpk[0:64, 512:576] = _EYE
    return cpk


_CPK = [_pack_phase1_consts(0), _pack_phase1_consts(1)]


# ----------------------------------------------------------------------------
# Phase 1: DCT-Y (contract ny, full) then rFFT-X (contract local nx half)
#   in : xt_in [512, 8192]  ny-major view of this core's x shard, bf16
#        cyt   [512, 64]    Cy^T (DCT lhsT)
#        fxt   [256, 128]   FxT rows for this nx-half
#        ident [64, 64]
#   out: xtr   [128, 2048]  [alpha, (j, m)] fp32  (partial: sum over h needed)
# ----------------------------------------------------------------------------
def build_phase1():
    nc = bass.Bass()
    xt_in = nc.declare_dram_parameter("xt_in", [NY, NXH * DV], BF16,
                                      isOutput=False)
    # packed consts: cols 0:256 cyt (c at c*64), 256:512 fxt (c2 at c2*128),
    # 512:576 identity (rows 0:64)
    cpk = nc.declare_dram_parameter("cpk", [128, 576], BF16, isOutput=False)
    xtr = nc.declare_dram_parameter("xtr", [128, DV * KY], F32, isOutput=True)

    with ExitStack() as ctx:
        tc = ctx.enter_context(tile.TileContext(nc))
        consts = ctx.enter_context(tc.tile_pool(name="consts", bufs=1))
        xpool = ctx.enter_context(tc.tile_pool(name="xpool", bufs=1))
        tpool = ctx.enter_context(tc.tile_pool(name="tpool", bufs=1))
        ttpool = ctx.enter_context(tc.tile_pool(name="ttpool", bufs=1))
        xtrpool = ctx.enter_context(tc.tile_pool(name="xtrpool", bufs=1))
        psD = ctx.enter_context(tc.tile_pool(name="psD", bufs=2, space="PSUM"))
        psT = ctx.enter_context(tc.tile_pool(name="psT", bufs=2, space="PSUM"))
        psF = ctx.enter_context(tc.tile_pool(name="psF", bufs=1, space="PSUM"))

        cpk_t = consts.tile([128, 576], BF16)
        nc.sync.dma_start(out=cpk_t, in_=cpk[:, :])
        id_t = cpk_t[0:64, 512:576]

        # x tiles [128 ny, 4096]: (c = ny chunk, gh = col half); DMA in use
        # order (gh major) so the first DCT group can start after one tile.
        xts = {}
        for gh in range(2):
            for c in range(4):
                xt = xpool.tile([128, 4096], BF16, tag=f"xt{c}_{gh}",
                                name=f"xt{c}_{gh}")
                nc.sync.dma_start(
                    out=xt,
                    in_=xt_in[c * 128:(c + 1) * 128,
                              gh * 4096:(gh + 1) * 4096])
                xts[(c, gh)] = xt

        T = tpool.tile([64, NXH * DV], BF16, tag="T", name="T")
        Tv = T.rearrange("p (nx j) -> p nx j", j=DV)
        TT = ttpool.tile([128, 2 * DV * KY], BF16, tag="TT", name="TT")
        xtr_s = xtrpool.tile([128, DV * KY], F32, tag="xtr", name="xtr_s")
        # FFT accumulators persist across both nx-half chunks
        pF = [psF.tile([128, 1024], F32, tag=f"pF{np_}", name=f"pF{np_}")
              for np_ in range(2)]

        # Emit per nx-half (gh == c2): DCT groups, transposes, FFT partials —
        # lets half 0's tail work overlap half 1's input DMA.
        for gh in range(2):
            # ---- stage DCT-Y: T[m 64, (nx 256, j 32)] = Cy @ x ----
            for gf in range(8):                      # 512-col fine groups
                off = gf * 512
                ps = psD.tile([64, 512], F32, tag="dct", name=f"dct{gh}_{gf}")
                for c in range(4):
                    nc.tensor.matmul(
                        ps, cpk_t[:, c * KY:(c + 1) * KY],
                        xts[(c, gh)][:, off:off + 512],
                        start=(c == 0), stop=(c == 3))
                _copy(nc, gf, T[:, gh * 4096 + off:gh * 4096 + off + 512], ps)

            # ---- transposes: TT[nx 128 (c2), (j 32, m 64)] = T^T per j ----
            c2 = gh
            for jg in range(4):
                pT = psT.tile([128, 512], BF16, tag="pT", name=f"pT{c2}_{jg}")
                for jj in range(8):
                    j = jg * 8 + jj
                    nc.tensor.transpose(
                        pT[:, jj * KY:(jj + 1) * KY],
                        Tv[:, c2 * 128:(c2 + 1) * 128, j], id_t)
                _copy(nc, jg, TT[:, c2 * 2048 + jg * 512:
                                 c2 * 2048 + (jg + 1) * 512], pT)

            # ---- stage rFFT-X partial (contract this nx chunk) ----
            for n in range(4):
                nc.tensor.matmul(
                    pF[n // 2][:, (n % 2) * 512:(n % 2 + 1) * 512],
                    cpk_t[:, 256 + c2 * 128:256 + (c2 + 1) * 128],
                    TT[:, c2 * 2048 + n * 512:c2 * 2048 + (n + 1) * 512],
                    start=(c2 == 0), stop=(c2 == 1))

        for n in range(4):
            _copy(nc, n, xtr_s[:, n * 512:(n + 1) * 512],
                  pF[n // 2][:, (n % 2) * 512:(n % 2 + 1) * 512])
            nc.sync.dma_start(out=xtr[:, n * 512:(n + 1) * 512],
                              in_=xtr_s[:, n * 512:(n + 1) * 512])
    return _split_multiwait(nc)


# ----------------------------------------------------------------------------
# Phase 2a: per-mode complex channel mix, sharded over a (8 a-values per core)
#   in : w2   [128, 256*64]  [(rr/ri, j), (g, i32)]  R slice, bf16
#        x2   [128, 256*8]   [(p, j), (g, q, b)] spectrum, bf16
#   out: y    [64, 8*64*4]   [(u, i), (g, q, b)] fp32
# ----------------------------------------------------------------------------
def build_phase2a():
    NMODE = (KX // NCORES) * KY                      # 512 modes per core
    NG = NMODE // 2                                  # 256 mode-pair groups
    nc = bass.Bass()
    w2 = nc.declare_dram_parameter("w2", [128, NG * 64], BF16, isOutput=False)
    x2 = nc.declare_dram_parameter("x2", [128, NG * 8], BF16, isOutput=False)
    y = nc.declare_dram_parameter("y", [64, NMODE * B], F32, isOutput=True)

    with ExitStack() as ctx:
        tc = ctx.enter_context(tile.TileContext(nc))
        consts = ctx.enter_context(tc.tile_pool(name="consts", bufs=1))
        outpool = ctx.enter_context(tc.tile_pool(name="outpool", bufs=1))
        psY = ctx.enter_context(tc.tile_pool(name="psY", bufs=4, space="PSUM"))

        x_ts = []
        for c in range(2):
            x_c = consts.tile([128, NG * 4], BF16, tag=f"x{c}", name=f"x{c}")
            nc.sync.dma_start(out=x_c, in_=x2[:, c * NG * 4:(c + 1) * NG * 4])
            x_ts.append(x_c)
        w_ts = []
        for c in range(8):
            w_c = consts.tile([128, 2048], BF16, tag=f"w{c}", name=f"w{c}")
            nc.sync.dma_start(out=w_c, in_=w2[:, c * 2048:(c + 1) * 2048])
            w_ts.append(w_c)
        y_ts = [outpool.tile([64, 512], F32, tag=f"y{bk}", name=f"y{bk}")
                for bk in range(4)]

        for bk in range(4):                          # 64 groups per psum bank
            pY = psY.tile([64, 512], F32)
            for gg in range(64):
                g = bk * 64 + gg
                nc.tensor.matmul(pY[:, gg * 8:(gg + 1) * 8],
                                 w_ts[g // 32][:, (g % 32) * 64:
                                               (g % 32 + 1) * 64],
                                 x_ts[g // 128][:, (g % 128) * 8:
                                                (g % 128 + 1) * 8],
                                 start=True, stop=True)
            _copy(nc, bk, y_ts[bk], pY)
            nc.sync.dma_start(out=y[:, bk * 512:(bk + 1) * 512], in_=y_ts[bk])
    return _split_multiwait(nc)


# ----------------------------------------------------------------------------
# Phase 2b: inverse transforms per (b, nx-half)
#   in : yb  [128, 2048]  [(q, a), (i, m)] bf16
#        gh  [128, 256]   G rows alpha, cols nx-local, bf16
#        cym [64, 512]    Cy [m, ny] bf16
#   out: oh2 [256, 16384] rows nx-local, cols (i, ny) bf16
# ----------------------------------------------------------------------------
def build_phase2b():
    nc = bass.Bass()
    # packed: cols 0:2048 yb [(q,a),(i,m)], 2048:2304 gh, 2304:2816 cym(r0:64)
    ypk = nc.declare_dram_parameter("ypk", [128, 2816], BF16, isOutput=False)
    oh2 = nc.declare_dram_parameter("oh2", [NXH, DV * NY], BF16, isOutput=True)

    with ExitStack() as ctx:
        tc = ctx.enter_context(tile.TileContext(nc))
        consts = ctx.enter_context(tc.tile_pool(name="consts", bufs=1))
        yrpool = ctx.enter_context(tc.tile_pool(name="yrpool", bufs=1))
        opool = ctx.enter_context(tc.tile_pool(name="opool", bufs=3))
        psD = ctx.enter_context(tc.tile_pool(name="psD", bufs=2, space="PSUM"))
        psE = ctx.enter_context(tc.tile_pool(name="psE", bufs=6, space="PSUM"))

        ypk_t = consts.tile([128, 2816], BF16)
        nc.sync.dma_start(out=ypk_t, in_=ypk[:, :])
        gh_t = ypk_t[:, 2048:2304]
        cym_t = ypk_t[0:64, 2304:2816]

        # stage D: yr_i [m64, nx256] = yb[:, i]^T @ gh
        YRs = [yrpool.tile([64, 2 * NXH], BF16, tag=f"YR{ip}", bufs=1,
                           name=f"YR{ip}") for ip in range(16)]  # [m,(i2,nx)]
        for ip in range(DV // 2):
            pD = psD.tile([64, 2 * NXH], F32)
            for ii in range(2):
                i = ip * 2 + ii
                nc.tensor.matmul(pD[:, ii * NXH:(ii + 1) * NXH],
                                 ypk_t[:, i * KY:(i + 1) * KY], gh_t,
                                 start=True, stop=True)
            _copy(nc, ip, YRs[ip], pD)

        # stage E: out[nx 128, ny 512] per (i, kc); dense drains into
        # [nx, (i, ny)] tiles (host re-transposes to [nx, ny, i] for free).
        for kc in range(2):
            for ig in range(4):                      # 8 i's per output tile
                Oh = opool.tile([128, 8 * NY], BF16, tag="Oh",
                                name=f"Oh{kc}_{ig}")
                for ii in range(8):
                    i = ig * 8 + ii
                    pE = psE.tile([128, NY], F32)
                    nc.tensor.matmul(pE,
                                     YRs[i // 2][:, (i % 2) * NXH + kc * 128:
                                                 (i % 2) * NXH + (kc + 1) * 128],
                                     cym_t, start=True, stop=True)
                    _copy(nc, i, Oh[:, ii * NY:(ii + 1) * NY], pE)
                nc.sync.dma_start(
                    out=oh2[kc * 128:(kc + 1) * 128,
                            ig * 8 * NY:(ig + 1) * 8 * NY],
                    in_=Oh)
    return _split_multiwait(nc)


_NC_CACHE = {}
LAST_EXEC_NS = []


def _get(name):
    if name not in _NC_CACHE:
        _NC_CACHE[name] = {"p1": build_phase1, "p2a": build_phase2a,
                           "p2b": build_phase2b}[name]()
    return _NC_CACHE[name]


def kernel(x, R_real, R_imag):
    x = np.ascontiguousarray(x, dtype=np.float32)
    AL = KX // NCORES

    # ---------------- phase 1 ----------------
    in1 = []
    for c in range(NCORES):
        b, h = c // 2, c % 2
        xh = x[b, h * NXH:(h + 1) * NXH]              # [256, 512, 32]
        xt = np.ascontiguousarray(xh.transpose(1, 0, 2)).astype(NPBF16)
        in1.append({
            "xt_in": xt.reshape(NY, NXH * DV),
            "cpk": _CPK[h],
        })
    LAST_EXEC_NS.clear()
    r1 = run_bass_kernel_spmd(_get("p1"), in1, list(range(NCORES)))
    LAST_EXEC_NS.append(r1.exec_time_ns)
    # partials [alpha, j, m] per (b, h); sum halves -> spect [B, 128, 32, 64]
    parts = [r1.results[c]["xtr"].reshape(128, DV, KY) for c in range(NCORES)]
    spect = np.stack([parts[2 * b] + parts[2 * b + 1] for b in range(B)])

    # ---------------- phase 2a ----------------
    NMODE = AL * KY
    NG = NMODE // 2
    in2 = []
    for s in range(NCORES):
        a_sl = slice(s * AL, (s + 1) * AL)
        # [j, i, mode] slices of R (mode = a_l*64 + m)
        Rr_t = R_real[:, :, a_sl, :].transpose(1, 0, 2, 3).reshape(DV, DV, NMODE)
        Ri_t = R_imag[:, :, a_sl, :].transpose(1, 0, 2, 3).reshape(DV, DV, NMODE)
        W2 = np.zeros((128, NG, 64), dtype=np.float32)
        # spect [B, alpha, j, m] -> xr/xi [j, mode, b]
        xr = spect[:, a_sl, :, :].transpose(2, 1, 3, 0).reshape(DV, NMODE, B)
        xi = (spect[:, 64 + s * AL:64 + (s + 1) * AL, :, :]
              .transpose(2, 1, 3, 0).reshape(DV, NMODE, B))
        X2 = np.empty((128, NG, 2, B), dtype=np.float32)
        for u in range(2):
            r0, r1_, r2_ = u * 64, u * 64 + 32, u * 64 + 64
            W2[r0:r1_, :, u * 32:(u + 1) * 32] = (
                Rr_t[:, :, u::2].transpose(0, 2, 1))
            W2[r1_:r2_, :, u * 32:(u + 1) * 32] = (
                Ri_t[:, :, u::2].transpose(0, 2, 1))
            X2[r0:r1_, :, 0, :] = xr[:, u::2, :]
            X2[r1_:r2_, :, 0, :] = -xi[:, u::2, :]
            X2[r0:r1_, :, 1, :] = xi[:, u::2, :]
            X2[r1_:r2_, :, 1, :] = xr[:, u::2, :]
        in2.append({"w2": W2.reshape(128, NG * 64).astype(NPBF16),
                    "x2": X2.reshape(128, NG * 8).astype(NPBF16)})
    r2 = run_bass_kernel_spmd(_get("p2a"), in2, list(range(NCORES)))
    LAST_EXEC_NS.append(r2.exec_time_ns)
    # y core result [64=(u,i), (g, q, b)] -> [q, i, a_l, m, b] per core
    ys = []
    for s in range(NCORES):
        t = r2.results[s]["y"].reshape(2, DV, NG, 2, B)       # [u, i, g, q, b]
        t = t.transpose(3, 1, 2, 0, 4).reshape(2, DV, NMODE, B)
        ys.append(t.reshape(2, DV, AL, KY, B))
    yfull = np.stack(ys)                                       # [s, q, i, a_l, m, b]
    yfull = yfull.transpose(1, 2, 0, 3, 4, 5).reshape(2, DV, KX, KY, B)

    # ---------------- phase 2b ----------------
    in3 = []
    for c in range(NCORES):
        b, h = c // 2, c % 2
        ypk = np.zeros((128, 2816), dtype=NPBF16)
        # yb [(q, a), (i, m)]
        ybc = yfull[:, :, :, :, b].transpose(0, 2, 1, 3).reshape(128, DV * KY)
        ypk[:, 0:2048] = ybc.astype(NPBF16)
        ypk[:, 2048:2304] = _G[:, h * NXH:(h + 1) * NXH]
        ypk[0:64, 2304:2816] = _CY
        in3.append({"ypk": ypk})
    r3 = run_bass_kernel_spmd(_get("p2b"), in3, list(range(NCORES)))
    LAST_EXEC_NS.append(r3.exec_time_ns)

    out = np.empty((B, NX, NY, DV), dtype=np.float32)
    for c in range(NCORES):
        b, h = c // 2, c % 2
        oh2 = r3.results[c]["oh2"].reshape(NXH, DV, NY)
        out[b, h * NXH:(h + 1) * NXH] = (
            oh2.transpose(0, 2, 1).astype(np.float32))
    return out


# revision 24
# speedup vs baseline: 1.8063x; 1.0434x over previous
"""Trainium2 Bass kernel for the truncated-spectrum 2D conv (CF2DConv).

Math: out = iDCT_y( irfft_x( mix_per_mode( rfft_x( DCT_y(x) )[:64,:64] ) ) )
All transforms are dense truncated matrices; the whole op is a chain of
matmuls plus a per-mode complex channel mix.

v2: bf16 data path (fp32 PSUM accumulation), DCT-before-FFT ordering in the
forward pass (truncates Y 512->64 before the X transform, cutting PE work
~2x), dense PSUM drains in the inverse pass with host-side final transpose.

Execution: 3 SPMD launches on 8 NeuronCores.
  phase 1  (shard (b, nx-half)): partial forward spectrum per core
  phase 2a (shard a-modes):      per-mode complex mix, R read exactly once
  phase 2b (shard (b, nx-half)): inverse transforms, dense output writes
Host does the (cheap, few-MB) re-shards between launches.
"""
import numpy as np
import ml_dtypes
from contextlib import ExitStack

import concourse.bass as bass
import concourse.mybir as mybir
import concourse.tile as tile
from concourse.bass_utils import run_bass_kernel_spmd

B, NX, NY, DV = 4, 512, 512, 32
KX, KY = 64, 64
NCORES = 8
NXH = NX // 2          # 256 rows per (b, h) core
F32 = mybir.dt.float32
BF16 = mybir.dt.bfloat16
NPBF16 = ml_dtypes.bfloat16


def _split_multiwait(nc):
    """Each 64B engine instruction has ONE sync-wait slot; Tile can attach
    several (e.g. two operands arriving on different DMAHW sem lanes), which
    walrus codegen rejects ("Too many sync wait commands"). Spill excess
    waits (and updates) onto chains of single-wait no-ops on the same
    engine queue."""
    cnt = 0
    for fn in nc.m.functions:
        for blk in fn.blocks:
            insts = list(blk.instructions)
            out = []
            changed = False
            for inst in insts:
                si = inst.sync_info
                if si is not None:
                    waits = list(si.on_wait or [])
                    ups = list(si.on_update or [])
                    if len(waits) > 1:
                        for w in waits[:-1]:
                            cnt += 1
                            out.append(mybir.InstNoOp(
                                name=f"premw{cnt}_{inst.name}",
                                sync_info=mybir.SyncInfo(on_wait=[w],
                                                         on_update=[]),
                                bass_nofuse=True, engine=inst.engine))
                        inst.sync_info = mybir.SyncInfo(
                            on_wait=waits[-1:], on_update=ups)
                        changed = True
                    if len(ups) > 1:
                        inst.sync_info = mybir.SyncInfo(
                            on_wait=list(inst.sync_info.on_wait or []),
                            on_update=ups[:1])
                        out.append(inst)
                        for u in ups[1:]:
                            cnt += 1
                            out.append(mybir.InstNoOp(
                                name=f"postmw{cnt}_{inst.name}",
                                sync_info=mybir.SyncInfo(on_wait=[],
                                                         on_update=[u]),
                                bass_nofuse=True, engine=inst.engine))
                        changed = True
                        continue
                out.append(inst)
            if changed:
                blk.instructions = out
    return nc


# ----------------------------------------------------------------------------
# Host-side constant transform matrices (float64 -> bf16)
# ----------------------------------------------------------------------------


def _copy(nc, idx, out, in_):
    if idx % 2 == 0:
        nc.scalar.copy(out, in_)
    else:
        nc.vector.tensor_copy(out, in_)


def _tcopy(nc, idx, out, psum_f32):
    """Drain fp32 PSUM to bf16 SBUF by copying the high half-words
    (truncation instead of RTNE): 16-bit copies run ~2x faster on DVE."""
    n = psum_f32.free_size()
    hi = psum_f32.bitcast(BF16).rearrange("p (n two) -> p n two", two=2)[:, :, 1]
    if idx % 2 == 0:
        nc.scalar.copy(out, hi)
    else:
        nc.vector.tensor_copy(out, hi)


def _build_consts():
    ny = np.arange(NY)
    m = np.arange(KY)
    Cy = np.cos(np.pi * (2 * ny[None, :] + 1) * m[:, None] / (2 * NY))
    s = np.full((KY, 1), np.sqrt(2.0 / NY)); s[0, 0] = np.sqrt(1.0 / NY)
    Cy = Cy * s                                     # [KY, NY]

    nx = np.arange(NX)
    a = np.arange(KX)
    ang = 2 * np.pi * a[:, None] * nx[None, :] / NX
    Fre = np.cos(ang) / np.sqrt(NX)                 # [KX, NX]
    Fim = -np.sin(ang) / np.sqrt(NX)

    w = np.full(KX, 2.0); w[0] = 1.0
    Gr = w[None, :] * np.cos(ang.T) / np.sqrt(NX)   # [NX, KX]
    Gi = -w[None, :] * np.sin(ang.T) / np.sqrt(NX)

    # FxT_all [NX, 128]: cols 0:64 = Fre^T, 64:128 = Fim^T
    FxT = np.concatenate([Fre.T, Fim.T], axis=1)    # [512, 128]
    CyT = Cy.T                                      # [512, 64] DCT lhsT
    # G_all [128(alpha), NX]: rows 0:64 = Gr^T, 64:128 = Gi^T
    G = np.concatenate([Gr.T, Gi.T], axis=0)        # [128, 512]
    return (FxT.astype(NPBF16), CyT.astype(NPBF16),
            G.astype(NPBF16), Cy.astype(NPBF16))


_FXT, _CYT, _G, _CY = _build_consts()
_EYE = np.eye(64, dtype=NPBF16)


def _pack_phase1_consts(h):
    cpk = np.zeros((128, 576), dtype=NPBF16)
    for c in range(4):
        cpk[:, c * 64:(c + 1) * 64] = _CYT[c * 128:(c + 1) * 128]
    fxt = _FXT[h * NXH:(h + 1) * NXH]
    for c2 in range(2):
        cpk[:, 256 + c2 * 128:256 + (c2 + 1) * 128] = (
            fxt[c2 * 128:(c2 + 1) * 128])
    cpk[0:64, 512:576] = _EYE
    return cpk


_CPK = [_pack_phase1_consts(0), _pack_phase1_consts(1)]


# ----------------------------------------------------------------------------
# Phase 1: DCT-Y (contract ny, full) then rFFT-X (contract local nx half)
#   in : xt_in [512, 8192]  ny-major view of this core's x shard, bf16
#        cyt   [512, 64]    Cy^T (DCT lhsT)
#        fxt   [256, 128]   FxT rows for this nx-half
#        ident [64, 64]
#   out: xtr   [128, 2048]  [alpha, (j, m)] fp32  (partial: sum over h needed)
# ----------------------------------------------------------------------------
def build_phase1():
    nc = bass.Bass()
    # x pre-packed on host as [128, (gf 16, c 4, 512)]: fine col-group gf's
    # four ny-chunk blocks are contiguous, so DMA arrival order matches the
    # DCT's consumption order.
    xt_in = nc.declare_dram_parameter("xt_in", [128, NY * NXH * DV // 128],
                                      BF16, isOutput=False)
    # packed consts: cols 0:256 cyt (c at c*64), 256:512 fxt (c2 at c2*128),
    # 512:576 identity (rows 0:64)
    cpk = nc.declare_dram_parameter("cpk", [128, 576], BF16, isOutput=False)
    xtr = nc.declare_dram_parameter("xtr", [128, DV * KY], F32, isOutput=True)

    with ExitStack() as ctx:
        tc = ctx.enter_context(tile.TileContext(nc))
        consts = ctx.enter_context(tc.tile_pool(name="consts", bufs=1))
        xpool = ctx.enter_context(tc.tile_pool(name="xpool", bufs=1))
        tpool = ctx.enter_context(tc.tile_pool(name="tpool", bufs=1))
        ttpool = ctx.enter_context(tc.tile_pool(name="ttpool", bufs=1))
        xtrpool = ctx.enter_context(tc.tile_pool(name="xtrpool", bufs=1))
        psD = ctx.enter_context(tc.tile_pool(name="psD", bufs=2, space="PSUM"))
        psT = ctx.enter_context(tc.tile_pool(name="psT", bufs=2, space="PSUM"))
        psF = ctx.enter_context(tc.tile_pool(name="psF", bufs=1, space="PSUM"))

        cpk_t = consts.tile([128, 576], BF16)
        nc.sync.dma_start(out=cpk_t, in_=cpk[:, :])
        id_t = cpk_t[0:64, 512:576]

        # x tiles [128, 4096]: tile t holds fine groups (2t, 2t+1) complete
        xts = []
        for t in range(8):
            xt = xpool.tile([128, 4096], BF16, tag=f"xt{t}", name=f"xt{t}")
            nc.sync.dma_start(out=xt,
                              in_=xt_in[:, t * 4096:(t + 1) * 4096])
            xts.append(xt)

        T = tpool.tile([64, NXH * DV], BF16, tag="T", name="T")
        Tv = T.rearrange("p (nx j) -> p nx j", j=DV)
        TT = ttpool.tile([128, 2 * DV * KY], BF16, tag="TT", name="TT")
        xtr_s = xtrpool.tile([128, DV * KY], F32, tag="xtr", name="xtr_s")
        # FFT accumulators persist across both nx-half chunks
        pF = [psF.tile([128, 1024], F32, tag=f"pF{np_}", name=f"pF{np_}")
              for np_ in range(2)]

        # Emit per nx-half (gh == c2): DCT groups, transposes, FFT partials —
        # lets half 0's tail work overlap half 1's input DMA.
        for gh in range(2):
            # ---- stage DCT-Y: T[m 64, (nx 256, j 32)] = Cy @ x ----
            for gf8 in range(8):                     # 512-col fine groups
                gf = gh * 8 + gf8
                ps = psD.tile([64, 512], F32, tag="dct", name=f"dct{gf}")
                for c in range(4):
                    nc.tensor.matmul(
                        ps, cpk_t[:, c * KY:(c + 1) * KY],
                        xts[gf // 2][:, (gf % 2) * 2048 + c * 512:
                                     (gf % 2) * 2048 + (c + 1) * 512],
                        start=(c == 0), stop=(c == 3))
                _tcopy(nc, gf, T[:, gf * 512:(gf + 1) * 512], ps)

            # ---- transposes: TT[nx 128 (c2), (j 32, m 64)] = T^T per j ----
            c2 = gh
            for jg in range(4):
                pT = psT.tile([128, 512], BF16, tag="pT", name=f"pT{c2}_{jg}")
                for jj in range(8):
                    j = jg * 8 + jj
                    nc.tensor.transpose(
                        pT[:, jj * KY:(jj + 1) * KY],
                        Tv[:, c2 * 128:(c2 + 1) * 128, j], id_t)
                _copy(nc, jg, TT[:, c2 * 2048 + jg * 512:
                                 c2 * 2048 + (jg + 1) * 512], pT)

            # ---- stage rFFT-X partial (contract this nx chunk) ----
            for n in range(4):
                nc.tensor.matmul(
                    pF[n // 2][:, (n % 2) * 512:(n % 2 + 1) * 512],
                    cpk_t[:, 256 + c2 * 128:256 + (c2 + 1) * 128],
                    TT[:, c2 * 2048 + n * 512:c2 * 2048 + (n + 1) * 512],
                    start=(c2 == 0), stop=(c2 == 1))

        for n in range(4):
            _copy(nc, n, xtr_s[:, n * 512:(n + 1) * 512],
                  pF[n // 2][:, (n % 2) * 512:(n % 2 + 1) * 512])
            nc.sync.dma_start(out=xtr[:, n * 512:(n + 1) * 512],
                              in_=xtr_s[:, n * 512:(n + 1) * 512])
    return _split_multiwait(nc)


# ----------------------------------------------------------------------------
# Phase 2a: per-mode complex channel mix, sharded over a (8 a-values per core)
#   in : w2   [128, 256*64]  [(rr/ri, j), (g, i32)]  R slice, bf16
#        x2   [128, 256*8]   [(p, j), (g, q, b)] spectrum, bf16
#   out: y    [64, 8*64*4]   [(u, i), (g, q, b)] fp32
# ----------------------------------------------------------------------------
def build_phase2a():
    NMODE = (KX // NCORES) * KY                      # 512 modes per core
    NG = NMODE // 2                                  # 256 mode-pair groups
    nc = bass.Bass()
    w2 = nc.declare_dram_parameter("w2", [128, NG * 64], BF16, isOutput=False)
    x2 = nc.declare_dram_parameter("x2", [128, NG * 8], BF16, isOutput=False)
    y = nc.declare_dram_parameter("y", [64, NMODE * B], F32, isOutput=True)

    with ExitStack() as ctx:
        tc = ctx.enter_context(tile.TileContext(nc))
        consts = ctx.enter_context(tc.tile_pool(name="consts", bufs=1))
        outpool = ctx.enter_context(tc.tile_pool(name="outpool", bufs=1))
        psY = ctx.enter_context(tc.tile_pool(name="psY", bufs=4, space="PSUM"))

        x_ts = []
        for c in range(2):
            x_c = consts.tile([128, NG * 4], BF16, tag=f"x{c}", name=f"x{c}")
            nc.sync.dma_start(out=x_c, in_=x2[:, c * NG * 4:(c + 1) * NG * 4])
            x_ts.append(x_c)
        w_ts = []
        for c in range(8):
            w_c = consts.tile([128, 2048], BF16, tag=f"w{c}", name=f"w{c}")
            nc.sync.dma_start(out=w_c, in_=w2[:, c * 2048:(c + 1) * 2048])
            w_ts.append(w_c)
        y_ts = [outpool.tile([64, 512], F32, tag=f"y{bk}", name=f"y{bk}")
                for bk in range(4)]

        for bk in range(4):                          # 64 groups per psum bank
            pY = psY.tile([64, 512], F32)
            for gg in range(64):
                g = bk * 64 + gg
                nc.tensor.matmul(pY[:, gg * 8:(gg + 1) * 8],
                                 w_ts[g // 32][:, (g % 32) * 64:
                                               (g % 32 + 1) * 64],
                                 x_ts[g // 128][:, (g % 128) * 8:
                                                (g % 128 + 1) * 8],
                                 start=True, stop=True)
            _copy(nc, bk, y_ts[bk], pY)
            nc.sync.dma_start(out=y[:, bk * 512:(bk + 1) * 512], in_=y_ts[bk])
    return _split_multiwait(nc)


# ----------------------------------------------------------------------------
# Phase 2b: inverse transforms per (b, nx-half)
#   in : yb  [128, 2048]  [(q, a), (i, m)] bf16
#        gh  [128, 256]   G rows alpha, cols nx-local, bf16
#        cym [64, 512]    Cy [m, ny] bf16
#   out: oh2 [256, 16384] rows nx-local, cols (i, ny) bf16
# ----------------------------------------------------------------------------
def build_phase2b():
    nc = bass.Bass()
    # packed: cols 0:2048 yb [(q,a),(i,m)], 2048:2304 gh, 2304:2816 cym(r0:64)
    ypk = nc.declare_dram_parameter("ypk", [128, 2816], BF16, isOutput=False)
    oh2 = nc.declare_dram_parameter("oh2", [NXH, DV * NY], BF16, isOutput=True)

    with ExitStack() as ctx:
        tc = ctx.enter_context(tile.TileContext(nc))
        consts = ctx.enter_context(tc.tile_pool(name="consts", bufs=1))
        yrpool = ctx.enter_context(tc.tile_pool(name="yrpool", bufs=1))
        opool = ctx.enter_context(tc.tile_pool(name="opool", bufs=3))
        psD = ctx.enter_context(tc.tile_pool(name="psD", bufs=2, space="PSUM"))
        psE = ctx.enter_context(tc.tile_pool(name="psE", bufs=6, space="PSUM"))

        ypk_t = consts.tile([128, 2816], BF16)
        nc.sync.dma_start(out=ypk_t, in_=ypk[:, :])
        gh_t = ypk_t[:, 2048:2304]
        cym_t = ypk_t[0:64, 2304:2816]

        # stage D: yr_i [m64, nx256] = yb[:, i]^T @ gh
        YRs = [yrpool.tile([64, 2 * NXH], BF16, tag=f"YR{ip}", bufs=1,
                           name=f"YR{ip}") for ip in range(16)]  # [m,(i2,nx)]
        for ip in range(DV // 2):
            pD = psD.tile([64, 2 * NXH], F32)
            for ii in range(2):
                i = ip * 2 + ii
                nc.tensor.matmul(pD[:, ii * NXH:(ii + 1) * NXH],
                                 ypk_t[:, i * KY:(i + 1) * KY], gh_t,
                                 start=True, stop=True)
            _tcopy(nc, ip, YRs[ip], pD)

        # stage E: out[nx 128, ny 512] per (i, kc); dense drains into
        # [nx, (i, ny)] tiles (host re-transposes to [nx, ny, i] for free).
        for kc in range(2):
            for ig in range(4):                      # 8 i's per output tile
                Oh = opool.tile([128, 8 * NY], BF16, tag="Oh",
                                name=f"Oh{kc}_{ig}")
                for ii in range(8):
                    i = ig * 8 + ii
                    pE = psE.tile([128, NY], F32)
                    nc.tensor.matmul(pE,
                                     YRs[i // 2][:, (i % 2) * NXH + kc * 128:
                                                 (i % 2) * NXH + (kc + 1) * 128],
                                     cym_t, start=True, stop=True)
                    _tcopy(nc, i, Oh[:, ii * NY:(ii + 1) * NY], pE)
                nc.sync.dma_start(
                    out=oh2[kc * 128:(kc + 1) * 128,
                            ig * 8 * NY:(ig + 1) * 8 * NY],
                    in_=Oh)
    return _split_multiwait(nc)


_NC_CACHE = {}
LAST_EXEC_NS = []


def _get(name):
    if name not in _NC_CACHE:
        _NC_CACHE[name] = {"p1": build_phase1, "p2a": build_phase2a,
                           "p2b": build_phase2b}[name]()
    return _NC_CACHE[name]


def kernel(x, R_real, R_imag):
    x = np.ascontiguousarray(x, dtype=np.float32)
    AL = KX // NCORES

    # ---------------- phase 1 ----------------
    in1 = []
    for c in range(NCORES):
        b, h = c // 2, c % 2
        xh = x[b, h * NXH:(h + 1) * NXH]              # [256, 512, 32]
        xt = xh.transpose(1, 0, 2).reshape(NY, NXH * DV)   # [ny, (nx, j)]
        # pack [(c 4, p 128) ny, (gf 16, 512) col] -> [p, (gf, c, 512)]
        xp = (xt.reshape(4, 128, 16, 512).transpose(1, 2, 0, 3)
              .reshape(128, NY * NXH * DV // 128))
        in1.append({
            "xt_in": np.ascontiguousarray(xp).astype(NPBF16),
            "cpk": _CPK[h],
        })
    LAST_EXEC_NS.clear()
    r1 = run_bass_kernel_spmd(_get("p1"), in1, list(range(NCORES)))
    LAST_EXEC_NS.append(r1.exec_time_ns)
    # partials [alpha, j, m] per (b, h); sum halves -> spect [B, 128, 32, 64]
    parts = [r1.results[c]["xtr"].reshape(128, DV, KY) for c in range(NCORES)]
    spect = np.stack([parts[2 * b] + parts[2 * b + 1] for b in range(B)])

    # ---------------- phase 2a ----------------
    NMODE = AL * KY
    NG = NMODE // 2
    in2 = []
    for s in range(NCORES):
        a_sl = slice(s * AL, (s + 1) * AL)
        # [j, i, mode] slices of R (mode = a_l*64 + m)
        Rr_t = R_real[:, :, a_sl, :].transpose(1, 0, 2, 3).reshape(DV, DV, NMODE)
        Ri_t = R_imag[:, :, a_sl, :].transpose(1, 0, 2, 3).reshape(DV, DV, NMODE)
        W2 = np.zeros((128, NG, 64), dtype=np.float32)
        # spect [B, alpha, j, m] -> xr/xi [j, mode, b]
        xr = spect[:, a_sl, :, :].transpose(2, 1, 3, 0).reshape(DV, NMODE, B)
        xi = (spect[:, 64 + s * AL:64 + (s + 1) * AL, :, :]
              .transpose(2, 1, 3, 0).reshape(DV, NMODE, B))
        X2 = np.empty((128, NG, 2, B), dtype=np.float32)
        for u in range(2):
            r0, r1_, r2_ = u * 64, u * 64 + 32, u * 64 + 64
            W2[r0:r1_, :, u * 32:(u + 1) * 32] = (
                Rr_t[:, :, u::2].transpose(0, 2, 1))
            W2[r1_:r2_, :, u * 32:(u + 1) * 32] = (
                Ri_t[:, :, u::2].transpose(0, 2, 1))
            X2[r0:r1_, :, 0, :] = xr[:, u::2, :]
            X2[r1_:r2_, :, 0, :] = -xi[:, u::2, :]
            X2[r0:r1_, :, 1, :] = xi[:, u::2, :]
            X2[r1_:r2_, :, 1, :] = xr[:, u::2, :]
        in2.append({"w2": W2.reshape(128, NG * 64).astype(NPBF16),
                    "x2": X2.reshape(128, NG * 8).astype(NPBF16)})
    r2 = run_bass_kernel_spmd(_get("p2a"), in2, list(range(NCORES)))
    LAST_EXEC_NS.append(r2.exec_time_ns)
    # y core result [64=(u,i), (g, q, b)] -> [q, i, a_l, m, b] per core
    ys = []
    for s in range(NCORES):
        t = r2.results[s]["y"].reshape(2, DV, NG, 2, B)       # [u, i, g, q, b]
        t = t.transpose(3, 1, 2, 0, 4).reshape(2, DV, NMODE, B)
        ys.append(t.reshape(2, DV, AL, KY, B))
    yfull = np.stack(ys)                                       # [s, q, i, a_l, m, b]
    yfull = yfull.transpose(1, 2, 0, 3, 4, 5).reshape(2, DV, KX, KY, B)

    # ---------------- phase 2b ----------------
    in3 = []
    for c in range(NCORES):
        b, h = c // 2, c % 2
        ypk = np.zeros((128, 2816), dtype=NPBF16)
        # yb [(q, a), (i, m)]
        ybc = yfull[:, :, :, :, b].transpose(0, 2, 1, 3).reshape(128, DV * KY)
        ypk[:, 0:2048] = ybc.astype(NPBF16)
        ypk[:, 2048:2304] = _G[:, h * NXH:(h + 1) * NXH]
        ypk[0:64, 2304:2816] = _CY
        in3.append({"ypk": ypk})
    r3 = run_bass_kernel_spmd(_get("p2b"), in3, list(range(NCORES)))
    LAST_EXEC_NS.append(r3.exec_time_ns)

    out = np.empty((B, NX, NY, DV), dtype=np.float32)
    for c in range(NCORES):
        b, h = c // 2, c % 2
        oh2 = r3.results[c]["oh2"].reshape(NXH, DV, NY)
        out[b, h * NXH:(h + 1) * NXH] = (
            oh2.transpose(0, 2, 1).astype(np.float32))
    return out
